# revision 14
# baseline (speedup 1.0000x reference)
"""Causal multi-head attention (B=2, T=2048, C=1024, H=16, D=64) on 8 trn2 cores.

Sharding: core c -> (batch c//4, head-group c%4 of 4 heads / 256 channels).
Each core computes q/k/v for its head group, causal attention, and a partial
output projection y_part[2048,1024] = attnout_g @ wo_g.T. The host sums the 4
per-group partials of each batch (the "all-reduce after wo" done host-side).

Device kernel (per core, SPMD identical program):
  phase B: xT,[wq|wk|wv]T loaded + rounded to float32r; q^T,k^T (head-channel
           major) and v (natural, with a ones column -> vaug) via PE matmuls.
  phase C: per head, per tq-tile(512): ST[tk128,tq512] = k^T.T @ q^T (K=64),
           P = exp(ST/8) on ScalarE (PSUM->SBUF, float32r), diagonal blocks
           multiplied by 0/1 masks, PV accumulated over tk into PSUM[65,512]
           where row 64 (ones column of vaug) is the softmax denominator;
           normalize with reciprocal + K=1 broadcast matmul.
  phase D: y[t,1024] = attnoutT.T @ woT, DMA'd out per 128-row tile.

All matmuls run in float32r (TF32-like, full PE rate at N>=256).
"""
import time
import hashlib
import numpy as np

import jax
import jax.numpy as jnp
from jax.sharding import Mesh, PartitionSpec
from jax.experimental.shard_map import shard_map

import concourse.bass as bass
import concourse.tile as tile
from concourse import bacc, mybir
from concourse import bass2jax
from concourse.bass2jax import _bass_exec_p, install_neuronx_cc_hook, partition_id_tensor

B, T, C = 2, 2048, 1024
H = 16
D = C // H            # 64
SCALE = D ** -0.5     # 0.125
N_CORES = 8
HG = H // (N_CORES // B)   # heads per core = 4
HC = HG * D                # channels per core = 256
KT = C // 128              # 8 contraction tiles
NT = T // 128              # 16 row tiles
NJ = T // 512              # 4 tq tiles
F32 = mybir.dt.float32
F32R = mybir.dt.float32r


# ---------------------------------------------------------------- device code

def _build_nc(block_info, n_uniq, loop_n=None, phases="BCD", cast_dma=False):
    """block_info[j][i] = None (skip) | -1 (full) | ("st", slot) (causal
    staircase applied as a PE matmul accumulation) | ("mk", idx>=0) (general
    mask tile multiplied on DVE).

    Inputs are declared float32r: the host pre-rounds to 11 mantissa bits
    (RNE), so plain HWDGE DMA loads land PE-ready with no casting pass.
    """
    n_shift = 1 + max([e[1] for row in block_info for e in row
                       if isinstance(e, tuple) and e[0] == "st"], default=-1)
    nc = bacc.Bacc("TRN2", target_bir_lowering=False, debug=False,
                   num_devices=N_CORES)
    xT_ap = nc.dram_tensor("xT", [C, T], F32R, kind="ExternalInput").ap()
    # weights host-packed so each SBUF partition's bytes are contiguous in
    # DRAM (8KB descriptors; scattered 1KB descriptors measured ~4x slower)
    wqT_ap = nc.dram_tensor("wqT", [128, KT * HC], F32R, kind="ExternalInput").ap()
    wkT_ap = nc.dram_tensor("wkT", [128, KT * HC], F32R, kind="ExternalInput").ap()
    wvT_ap = nc.dram_tensor("wvT", [128, KT * HC], F32R, kind="ExternalInput").ap()
    woT_ap = nc.dram_tensor("woT", [128, 2 * C], F32R, kind="ExternalInput").ap()
    if n_uniq > 0:
        mk_ap = nc.dram_tensor("mk", [128, n_uniq * 512], F32,
                               kind="ExternalInput").ap()
    if n_shift > 0:
        # staircase mask as rank-structured matmul operands: score psum gets
        # triA.T @ bsh accumulated on top (adds -1e5 to causally-invalid
        # entries before the exp, replacing the DVE mask multiply)
        triA_ap = nc.dram_tensor("triA", [128, 128], F32R,
                                 kind="ExternalInput").ap()
        bsh_ap = nc.dram_tensor("bsh", [128, n_shift * 512], F32R,
                                kind="ExternalInput").ap()
    y_ap = nc.dram_tensor("y", [T, C], F32, kind="ExternalOutput").ap()

    with tile.TileContext(nc) as tc:
        with (
            nc.allow_low_precision(reason="float32r (tf32-like) matmul pipeline"),
            tc.tile_pool(name="glob", bufs=1) as pg,
            tc.tile_pool(name="warm", bufs=1) as pwarm,
        ):
            # persistent across phases
            qT = pg.tile([128, 2, T], F32R)        # [o-part, o-tile, t]
            # k^T zero-padded per head: head h lives in partition rows
            # 64*(h%2)..+64 of kTz[:, h, :], other rows stay 0 so the score
            # matmul runs at K=128 (fp32r is slow for K<128).
            kTz = pg.tile([128, HG, T], F32R)
            # v natural per tk-tile/head, padded to 128 cols: [v | 1 | zeros]
            # (fp32r is slow for M<128; ones column gives softmax denom).
            vaug = pg.tile([128, NT, HG, 128], F32R)
            ones128 = pg.tile([128, 128], F32R)    # all-ones lhsT for bcast
            recipz = pg.tile([128, 512], F32R)     # row0=recip, rows1-127 zero
            ident = pg.tile([128, 128], F32)       # PE-transpose identity
            if n_shift > 0:
                triA = pg.tile([128, 128], F32R)
                bsh = pg.tile([128, n_shift, 512], F32R)
                nc.sync.dma_start(triA[:], triA_ap)
                nc.sync.dma_start(
                    bsh[:], bsh_ap.rearrange("p (s f) -> p s f", f=512))

            # warm the Exp table while DMAs run; f32r rejects memset, so
            # zeros/ones are produced via rounding copies from f32 tiles.
            wtile = pwarm.tile([1, 16], F32)
            nc.vector.memset(wtile[:], 0.0)
            nc.scalar.activation(wtile[:], wtile[:],
                                 mybir.ActivationFunctionType.Exp)
            zt = pwarm.tile([128, 512], F32)
            nc.vector.memset(zt[:], 0.0)
            nc.vector.tensor_copy(kTz[:].rearrange("p h (j f) -> p h j f", f=512), zt[:, None, None, :].broadcast_to([128, HG, NJ, 512]))
            nc.vector.tensor_copy(vaug[:], zt[:, None, None, 0:128].broadcast_to([128, NT, HG, 128]))
            nc.vector.tensor_copy(recipz[:], zt[:])
            import concourse.masks as _masks
            _masks.make_identity(nc, ident[:])
            ot = pwarm.tile([128, 16], F32)
            nc.vector.memset(ot[:], 1.0)
            nc.vector.tensor_copy(vaug[:, :, :, D], ot[:, None, 0:HG].broadcast_to([128, NT, HG]))
            nc.vector.tensor_copy(ones128[:], ot[:, 0:1].broadcast_to([128, 128]))

            def body():
                # ---------------- phase A/B: loads + projections ------------
                with (
                    tc.tile_pool(name="ab", bufs=1) as pab,
                    tc.tile_pool(name="wp", bufs=2) as pwp,
                    tc.tile_pool(name="psq", bufs=4, space="PSUM") as psq,
                    tc.tile_pool(name="psv", bufs=3, space="PSUM") as psv,
                ):
                    xT = pab.tile([128, KT, T], F32R)
                    vT = pab.tile([128, 2, T], F32R)
                    # chunked so projections start as soon as chunk 0 lands
                    for kc in range(KT):
                        nc.sync.dma_start(
                            xT[:, kc, :],
                            xT_ap.rearrange("(k p) t -> k p t", p=128)[kc])
                    if "B" not in phases:
                        return

                    # q^T, k^T, v^T: [o, t] = w_g @ x^T ; kc-outer
                    for w_ap, qk in ((wqT_ap, 0), (wkT_ap, 1), (wvT_ap, 2)):
                        w_t = pwp.tile([128, KT, HC], F32R, tag="w", name=f"w{qk}")
                        nc.sync.dma_start(w_t[:], w_ap.rearrange("p (k m) -> p k m", k=KT))
                        for m in range(2):
                            pss = [psq.tile([128, 512], F32, tag="qkps", name=f"qkps{m}_{j}") for j in range(NJ)]
                            for kc in range(KT):
                                for j in range(NJ):
                                    nc.tensor.matmul(
                                        pss[j][:],
                                        w_t[:, kc, 128 * m:128 * (m + 1)],
                                        xT[:, kc, 512 * j:512 * (j + 1)],
                                        start=(kc == 0), stop=(kc == KT - 1))
                            for j in range(NJ):
                                sl = slice(512 * j, 512 * (j + 1))
                                if qk == 0:
                                    nc.scalar.copy(qT[:, m, sl], pss[j][:])
                                elif qk == 2:
                                    nc.scalar.copy(vT[:, m, sl], pss[j][:])
                                else:
                                    # scatter psum head-halves into kTz rows
                                    nc.scalar.copy(kTz[0:64, 2 * m, sl],
                                                   pss[j][0:64, :])
                                    nc.scalar.copy(kTz[64:128, 2 * m + 1, sl],
                                                   pss[j][64:128, :])
                    # v natural via PE transpose of vT 128x128 blocks
                    for m in range(2):
                        for i in range(NT):
                            ps = psv.tile([128, 128], F32, tag="vtp", name=f"vtp{m}_{i}")
                            nc.tensor.transpose(
                                ps[:], vT[:, m, 128 * i:128 * (i + 1)].bitcast(F32),
                                ident[:])
                            nc.vector.tensor_copy(
                                vaug[:, i, 2 * m:2 * m + 2, 0:D],
                                ps[:].rearrange("p (h d) -> p h d", h=2))

                # -------- phase C+D: attention, interleaved with out-proj ----
                if "C" not in phases and "D" not in phases:
                    return
                with (
                    tc.tile_pool(name="cd", bufs=1) as pcd,
                    tc.tile_pool(name="pt", bufs=4) as ppt,
                    tc.tile_pool(name="small", bufs=4) as psm,
                    tc.tile_pool(name="ys", bufs=3) as pys,
                    tc.tile_pool(name="psst", bufs=2, space="PSUM") as psst,
                    tc.tile_pool(name="pspv", bufs=2, space="PSUM") as pspv,
                    tc.tile_pool(name="psy", bufs=2, space="PSUM") as psy,
                ):
                    if n_uniq > 0:
                        mks = pcd.tile([128, n_uniq, 512], F32)
                        nc.sync.dma_start(mks[:], mk_ap.rearrange("p (u f) -> p u f", f=512))
                    woT = pcd.tile([128, 2, C], F32R)
                    nc.sync.dma_start(woT[:], woT_ap.rearrange("p (k m) -> p k m", k=2))
                    attnoutT = pcd.tile([128, 2, T], F32R)

                    for j in range(NJ):
                        blocks = [(i, bi) for i, bi in enumerate(block_info[j])
                                  if bi is not None]
                        chunks = [blocks[c:c + 2] for c in range(0, len(blocks), 2)]
                        for h in range(HG if "C" in phases else 0):
                            m = h // 2
                            jsl = slice(512 * j, 512 * (j + 1))
                            pv = pspv.tile([128, 512], F32, tag="pv", name=f"pv{h}_{j}")
                            n_acc = len(blocks)
                            acc = 0
                            prev_chunk = None  # (pt, idxs)

                            def emit_pv(pt, idxs):
                                nonlocal acc
                                for c, i in enumerate(idxs):
                                    nc.tensor.matmul(
                                        pv[:], vaug[:, i, h, :], pt[:, c, :],
                                        start=(acc == 0), stop=(acc == n_acc - 1))
                                    acc += 1

                            for ch in chunks:
                                nsub = len(ch)
                                st = psst.tile([128, 2, 512], F32, tag="st", name=f"st{h}_{j}")
                                for c, (i, bi) in enumerate(ch):
                                    stair = (isinstance(bi, tuple)
                                             and bi[0] == "st")
                                    nc.tensor.matmul(
                                        st[:, c, :],
                                        kTz[:, h, 128 * i:128 * (i + 1)],
                                        qT[:, m, jsl],
                                        start=True, stop=not stair)
                                    if stair:
                                        # add -1e5 to causally-invalid slots
                                        nc.tensor.matmul(
                                            st[:, c, :], triA[:],
                                            bsh[:, bi[1], :],
                                            start=False, stop=True)
                                pt = ppt.tile([128, 2, 512], F32R, tag="pt")
                                # one exp per chunk: the ~0.9us fixed per-op
                                # ACT cost dominates, so amortize over 1024
                                nc.scalar.activation(
                                    pt[:, 0:nsub, :], st[:, 0:nsub, :],
                                    mybir.ActivationFunctionType.Exp, scale=SCALE)
                                mi = [bi for _, bi in ch]
                                for c, b in enumerate(mi):
                                    if isinstance(b, tuple) and b[0] == "mk":
                                        nc.vector.tensor_mul(
                                            pt[:, c, :], pt[:, c, :],
                                            mks[:, b[1], :])
                                if prev_chunk is not None:
                                    emit_pv(*prev_chunk)
                                prev_chunk = (pt, [i for i, _ in ch])
                            emit_pv(*prev_chunk)
                            # normalization: 1/denom (row 64) broadcast down
                            # 128 partitions via ones-column matmul, then one
                            # fused psum*psum multiply into attnoutT
                            recip = psm.tile([1, 512], F32R, tag="recip")
                            nc.vector.reciprocal(recip[:], pv[64:65, :])
                            nc.vector.tensor_copy(recipz[0:1, :], recip[:])
                            bc = psy.tile([128, 512], F32, tag="yps", name=f"bc{h}_{j}")
                            nc.tensor.matmul(bc[:], ones128[:], recipz[:],
                                             start=True, stop=True)
                            avu = psm.tile([64, 512], F32, tag="avu")
                            nc.vector.tensor_copy(avu[:], pv[0:64, :])
                            row = 64 * (h % 2)
                            nc.vector.tensor_mul(
                                attnoutT[row:row + 64, m, jsl],
                                avu[:], bc[0:64, :])

                        # ---- phase D for this j: y rows [512j, 512j+512) ----
                        if "D" not in phases:
                            continue
                        for tp in range(2):     # pairs of row tiles
                            ys = pys.tile([128, 2, C], F32, tag="ys")
                            for tsub in range(2):
                                t = 4 * j + 2 * tp + tsub
                                for o2 in range(2):
                                    ps = psy.tile([128, 512], F32, tag="yps", name=f"yps{t}_{o2}")
                                    for kc in range(2):
                                        nc.tensor.matmul(
                                            ps[:],
                                            attnoutT[:, kc, 128 * t:128 * (t + 1)],
                                            woT[:, kc, 512 * o2:512 * (o2 + 1)],
                                            start=(kc == 0), stop=(kc == 1))
                                    nc.vector.tensor_copy(
                                        ys[:, tsub, 512 * o2:512 * (o2 + 1)], ps[:])
                            r0 = 512 * j + 256 * tp
                            nc.scalar.dma_start(
                                y_ap[r0:r0 + 256, :].rearrange("(tt p) o -> p tt o", p=128),
                                ys[:])

            if loop_n is None:
                body()
            else:
                with tc.For_i(0, loop_n, 1):
                    body()

    nc.compile()
    return nc


# ---------------------------------------------------------------- run harness

def _install_verbose_hook():
    install_neuronx_cc_hook()
    try:
        import libneuronxla
    except ImportError:
        return
    import traceback
    inner = bass2jax.neuronx_cc_hook

    def wrapped(*a, **kw):
        try:
            return inner(*a, **kw)
        except BaseException:
            traceback.print_exc()
            raise
    libneuronxla.neuronx_cc = wrapped


class _SpmdRunner:
    def __init__(self, nc, n_cores):
        _install_verbose_hook()
        self.nc, self.n_cores = nc, n_cores
        pname = nc.partition_id_tensor.name if nc.partition_id_tensor else None
        in_names, out_names, out_avals = [], [], []
        for alloc in nc.m.functions[0].allocations:
            if not isinstance(alloc, mybir.MemoryLocationSet):
                continue
            name = alloc.memorylocations[0].name
            if alloc.kind == "ExternalInput":
                if name != pname:
                    in_names.append(name)
            elif alloc.kind == "ExternalOutput":
                out_names.append(name)
                out_avals.append(jax.core.ShapedArray(
                    tuple(alloc.tensor_shape), mybir.dt.np(alloc.dtype)))
        self.in_names, self.out_names, self.out_avals = in_names, out_names, out_avals
        n_params = len(in_names)
        all_in = list(in_names) + list(out_names)
        if pname is not None:
            all_in.append(pname)

        def _body(*args):
            operands = list(args)
            if pname is not None:
                operands.append(partition_id_tensor())
            return tuple(_bass_exec_p.bind(
                *operands,
                out_avals=tuple(out_avals), in_names=tuple(all_in),
                out_names=tuple(out_names), lowering_input_output_aliases=(),
                sim_require_finite=True, sim_require_nnan=True, nc=nc))

        devices = jax.devices()[:n_cores]
        self.mesh = Mesh(np.asarray(devices), ("core",))
        in_specs = (PartitionSpec("core"),) * (n_params + len(out_names))
        out_specs = (PartitionSpec("core"),) * len(out_names)
        self.fn = jax.jit(shard_map(_body, mesh=self.mesh, in_specs=in_specs,
                                    out_specs=out_specs, check_rep=False),
                          keep_unused=True)
        self._shard = jax.sharding.NamedSharding(self.mesh, PartitionSpec("core"))

    def put_inputs(self, in_maps):
        arrs = []
        for name in self.in_names:
            cat = np.concatenate([np.asarray(m[name]) for m in in_maps], axis=0)
            arrs.append(jax.device_put(cat, self._shard))
        for av in self.out_avals:
            z = np.zeros((self.n_cores * av.shape[0], *av.shape[1:]), av.dtype)
            arrs.append(jax.device_put(z, self._shard))
        return arrs

    def run(self, dev_args):
        outs = self.fn(*dev_args)
        jax.block_until_ready(outs)
        return outs

    def results(self, outs):
        per_core = []
        for c in range(self.n_cores):
            per_core.append({
                name: np.asarray(outs[i]).reshape(
                    self.n_cores, *self.out_avals[i].shape)[c]
                for i, name in enumerate(self.out_names)})
        return per_core


# ---------------------------------------------------------------- host side

def _mask_blocks(mask):
    """Classify transposed 128x512 blocks of the [T,T] mask.

    Returns (block_info, uniq, shifts) where block_info[j][i] is None (all
    masked), -1 (all valid), ("st", slot) (causal staircase valid = p <=
    f - shifts[slot], applied on-device as a matmul accumulation), or
    ("mk", idx) (arbitrary mixed pattern, multiplied from uniq[idx]).
    """
    m2 = np.asarray(mask).reshape(T, T)
    valid = (m2 != -np.inf)          # [tq, tk]
    validT = valid.T                 # [tk, tq]
    uniq, keys = [], {}
    shifts, shift_keys = [], {}
    p_idx = np.arange(128)[:, None]
    f_idx = np.arange(512)[None, :]
    block_info = []
    for j in range(NJ):
        row = []
        for i in range(NT):
            blk = validT[128 * i:128 * (i + 1), 512 * j:512 * (j + 1)]
            if not blk.any():
                row.append(None)
                continue
            if blk.all():
                row.append(-1)
                continue
            s = 128 * i - 512 * j
            if -512 < s < 512 and np.array_equal(blk, p_idx <= f_idx - s):
                if s not in shift_keys:
                    shift_keys[s] = len(shifts)
                    shifts.append(s)
                row.append(("st", shift_keys[s]))
                continue
            k = hashlib.sha1(np.ascontiguousarray(blk)).hexdigest()
            if k not in keys:
                keys[k] = len(uniq)
                uniq.append(blk.astype(np.float32))
            row.append(("mk", keys[k]))
        block_info.append(row)
    return block_info, uniq, shifts


_MASK_BIG = -1.0e5


def _stair_operands(shifts):
    """triA [128,128] and bsh [128, n_shift, 512] with
    (triA.T @ bsh[:, slot])[p, f] = -1e5 where p > f - shifts[slot] else 0."""
    k_i = np.arange(128)
    p_i = np.arange(128)
    triA = ((p_i[None, :] > k_i[:, None]) | (k_i[:, None] == 127)
            ).astype(np.float32)
    bsh = np.zeros((128, len(shifts), 512), np.float32)
    for slot, s in enumerate(shifts):
        for f in range(512):
            g = f - s
            if g < 0:
                bsh[127, slot, f] = _MASK_BIG
            elif g <= 126:
                bsh[g, slot, f] = _MASK_BIG
    return triA, bsh


_CACHE = {}


def _get_runner(block_info, n_uniq, loop_n=None, phases="BCD", cast_dma=True):
    key = (str(block_info), n_uniq, loop_n, phases, cast_dma)
    if key not in _CACHE:
        nc = _build_nc(block_info, n_uniq, loop_n=loop_n, phases=phases, cast_dma=cast_dma)
        _CACHE[key] = _SpmdRunner(nc, N_CORES)
    return _CACHE[key]


def _round_f32r(a):
    """Round fp32 to f32r (RNE to 11 mantissa bits) so device DMA loads are
    PE-ready without a casting pass."""
    x = np.ascontiguousarray(a, np.float32).view(np.uint32).astype(np.uint64)
    half = np.uint64(0x7FF)
    out = (x + half + ((x >> np.uint64(12)) & np.uint64(1))) & np.uint64(0xFFFFF000)
    return out.astype(np.uint32).view(np.float32)


def _pack_rows(a):
    """[R*128, F] -> [128, R*F]: partition-contiguous packing for fast DMA."""
    r = a.shape[0] // 128
    return np.ascontiguousarray(
        a.reshape(r, 128, a.shape[1]).transpose(1, 0, 2).reshape(128, -1))


def _make_in_maps(x, mask, wq, wk, wv, wo):
    block_info, uniq, shifts = _mask_blocks(mask)
    x = np.asarray(x, np.float32)
    extra = {}
    if uniq:
        mk = np.stack(uniq)    # [u,128,512] -> [128, u*512]
        extra["mk"] = np.ascontiguousarray(
            mk.transpose(1, 0, 2).reshape(128, -1))
    if shifts:
        triA, bsh = _stair_operands(shifts)
        extra["triA"] = triA
        extra["bsh"] = np.ascontiguousarray(bsh.reshape(128, -1))
    in_maps = []
    for c in range(N_CORES):
        b, g = c // 4, c % 4
        sl = slice(HC * g, HC * (g + 1))
        in_maps.append({
            "xT": _round_f32r(x[b].T),
            "wqT": _pack_rows(_round_f32r(np.asarray(wq)[sl, :].T)),
            "wkT": _pack_rows(_round_f32r(np.asarray(wk)[sl, :].T)),
            "wvT": _pack_rows(_round_f32r(np.asarray(wv)[sl, :].T)),
            "woT": _pack_rows(_round_f32r(np.asarray(wo)[:, sl].T)),
            **extra,
        })
    return in_maps, block_info, len(uniq)


def kernel(x, mask, wq, wk, wv, wo):
    in_maps, block_info, n_uniq = _make_in_maps(x, mask, wq, wk, wv, wo)
    runner = _get_runner(block_info, n_uniq)
    dev = runner.put_inputs(in_maps)
    res = runner.results(runner.run(dev))
    out = np.zeros((B, T, C), np.float32)
    for c in range(N_CORES):
        out[c // 4] += res[c]["y"]
    return out



# revision 16
# speedup vs baseline: 1.1012x; 1.1012x over previous
"""Causal multi-head attention (B=2, T=2048, C=1024, H=16, D=64) on 8 trn2 cores.

Sharding: core c -> (batch c//4, head-group c%4 of 4 heads / 256 channels).
Each core computes q/k/v for its head group, causal attention, and a partial
output projection y_part[2048,1024] = attnout_g @ wo_g.T. The host sums the 4
per-group partials of each batch (the "all-reduce after wo" done host-side).

Device kernel (per core, SPMD identical program):
  phase B: xT,[wq|wk|wv]T loaded + rounded to float32r; q^T,k^T (head-channel
           major) and v (natural, with a ones column -> vaug) via PE matmuls.
  phase C: per head, per tq-tile(512): ST[tk128,tq512] = k^T.T @ q^T (K=64),
           P = exp(ST/8) on ScalarE (PSUM->SBUF, float32r), diagonal blocks
           multiplied by 0/1 masks, PV accumulated over tk into PSUM[65,512]
           where row 64 (ones column of vaug) is the softmax denominator;
           normalize with reciprocal + K=1 broadcast matmul.
  phase D: y[t,1024] = attnoutT.T @ woT, DMA'd out per 128-row tile.

All matmuls run in float32r (TF32-like, full PE rate at N>=256).
"""
import time
import hashlib
import numpy as np

import jax
import jax.numpy as jnp
from jax.sharding import Mesh, PartitionSpec
from jax.experimental.shard_map import shard_map

import concourse.bass as bass
import concourse.tile as tile
from concourse import bacc, mybir
from concourse import bass2jax
from concourse.bass2jax import _bass_exec_p, install_neuronx_cc_hook, partition_id_tensor

B, T, C = 2, 2048, 1024
H = 16
D = C // H            # 64
SCALE = D ** -0.5     # 0.125
N_CORES = 8
HG = H // (N_CORES // B)   # heads per core = 4
HC = HG * D                # channels per core = 256
KT = C // 128              # 8 contraction tiles
NT = T // 128              # 16 row tiles
NJ = T // 512              # 4 tq tiles
F32 = mybir.dt.float32
F32R = mybir.dt.float32r


# ---------------------------------------------------------------- device code

def _build_nc(block_info, n_uniq, loop_n=None, phases="BCD", cast_dma=False):
    """block_info[j][i] = None (skip) | -1 (full) | ("st", slot) (causal
    staircase applied as a PE matmul accumulation) | ("mk", idx>=0) (general
    mask tile multiplied on DVE).

    Inputs are declared float32r: the host pre-rounds to 11 mantissa bits
    (RNE), so plain HWDGE DMA loads land PE-ready with no casting pass.
    """
    n_shift = 1 + max([e[1] for row in block_info for e in row
                       if isinstance(e, tuple) and e[0] == "st"], default=-1)
    nc = bacc.Bacc("TRN2", target_bir_lowering=False, debug=False,
                   num_devices=N_CORES)
    xT_ap = nc.dram_tensor("xT", [C, T], F32R, kind="ExternalInput").ap()
    # weights host-packed so each SBUF partition's bytes are contiguous in
    # DRAM (8KB descriptors; scattered 1KB descriptors measured ~4x slower)
    wqT_ap = nc.dram_tensor("wqT", [128, KT * HC], F32R, kind="ExternalInput").ap()
    wkT_ap = nc.dram_tensor("wkT", [128, KT * HC], F32R, kind="ExternalInput").ap()
    wvT_ap = nc.dram_tensor("wvT", [128, KT * HC], F32R, kind="ExternalInput").ap()
    woT_ap = nc.dram_tensor("woT", [128, 2 * C], F32R, kind="ExternalInput").ap()
    if n_uniq > 0:
        mk_ap = nc.dram_tensor("mk", [128, n_uniq * 512], F32,
                               kind="ExternalInput").ap()
    if n_shift > 0:
        # staircase mask as rank-structured matmul operands: score psum gets
        # triA.T @ bsh accumulated on top (adds -1e5 to causally-invalid
        # entries before the exp, replacing the DVE mask multiply)
        triA_ap = nc.dram_tensor("triA", [128, 128], F32R,
                                 kind="ExternalInput").ap()
        bsh_ap = nc.dram_tensor("bsh", [128, n_shift * 512], F32R,
                                kind="ExternalInput").ap()
    y_ap = nc.dram_tensor("y", [T, C], F32, kind="ExternalOutput").ap()

    with tile.TileContext(nc) as tc:
        with (
            nc.allow_low_precision(reason="float32r (tf32-like) matmul pipeline"),
            tc.tile_pool(name="glob", bufs=1) as pg,
            tc.tile_pool(name="warm", bufs=1) as pwarm,
        ):
            # persistent across phases
            qT = pg.tile([128, 2, T], F32R)        # [o-part, o-tile, t]
            # k^T zero-padded per head: head h lives in partition rows
            # 64*(h%2)..+64 of kTz[:, h, :], other rows stay 0 so the score
            # matmul runs at K=128 (fp32r is slow for K<128).
            kTz = pg.tile([128, HG, T], F32R)
            # v natural per tk-tile/head, padded to 128 cols: [v | 1 | zeros]
            # (fp32r is slow for M<128; ones column gives softmax denom).
            vaug = pg.tile([128, NT, HG, 128], F32R)
            ones128 = pg.tile([128, 128], F32R)    # all-ones lhsT for bcast
            recipz = pg.tile([128, 512], F32R)     # row0=recip, rows1-127 zero
            ident = pg.tile([128, 128], F32)       # PE-transpose identity
            if n_shift > 0:
                triA = pg.tile([128, 128], F32R)
                bsh = pg.tile([128, n_shift, 512], F32R)
                nc.sync.dma_start(triA[:], triA_ap)
                nc.sync.dma_start(
                    bsh[:], bsh_ap.rearrange("p (s f) -> p s f", f=512))

            # warm the Exp table while DMAs run; f32r rejects memset, so
            # zeros/ones are produced via rounding copies from f32 tiles.
            wtile = pwarm.tile([1, 16], F32)
            nc.vector.memset(wtile[:], 0.0)
            nc.scalar.activation(wtile[:], wtile[:],
                                 mybir.ActivationFunctionType.Exp)
            zt = pwarm.tile([128, 512], F32)
            nc.vector.memset(zt[:], 0.0)
            nc.vector.tensor_copy(kTz[:].rearrange("p h (j f) -> p h j f", f=512), zt[:, None, None, :].broadcast_to([128, HG, NJ, 512]))
            nc.vector.tensor_copy(vaug[:], zt[:, None, None, 0:128].broadcast_to([128, NT, HG, 128]))
            nc.vector.tensor_copy(recipz[:], zt[:])
            import concourse.masks as _masks
            _masks.make_identity(nc, ident[:])
            ot = pwarm.tile([128, 16], F32)
            nc.vector.memset(ot[:], 1.0)
            nc.vector.tensor_copy(vaug[:, :, :, D], ot[:, None, 0:HG].broadcast_to([128, NT, HG]))
            nc.vector.tensor_copy(ones128[:], ot[:, 0:1].broadcast_to([128, 128]))

            def body():
                # ---------------- phase A/B: loads + projections ------------
                with (
                    tc.tile_pool(name="ab", bufs=1) as pab,
                    tc.tile_pool(name="wp", bufs=2) as pwp,
                    tc.tile_pool(name="psq", bufs=4, space="PSUM") as psq,
                    tc.tile_pool(name="psv", bufs=3, space="PSUM") as psv,
                ):
                    xT = pab.tile([128, KT, T], F32R)
                    vT = pab.tile([128, 2, T], F32R)
                    # chunked so projections start as soon as chunk 0 lands
                    for kc in range(KT):
                        nc.sync.dma_start(
                            xT[:, kc, :],
                            xT_ap.rearrange("(k p) t -> k p t", p=128)[kc])
                    if "B" not in phases:
                        return

                    # q^T, k^T, v^T: [o, t] = w_g @ x^T ; kc-outer
                    for w_ap, qk in ((wqT_ap, 0), (wkT_ap, 1), (wvT_ap, 2)):
                        w_t = pwp.tile([128, KT, HC], F32R, tag="w", name=f"w{qk}")
                        nc.sync.dma_start(w_t[:], w_ap.rearrange("p (k m) -> p k m", k=KT))
                        for m in range(2):
                            pss = [psq.tile([128, 512], F32, tag="qkps", name=f"qkps{m}_{j}") for j in range(NJ)]
                            for kc in range(KT):
                                for j in range(NJ):
                                    nc.tensor.matmul(
                                        pss[j][:],
                                        w_t[:, kc, 128 * m:128 * (m + 1)],
                                        xT[:, kc, 512 * j:512 * (j + 1)],
                                        start=(kc == 0), stop=(kc == KT - 1))
                            for j in range(NJ):
                                sl = slice(512 * j, 512 * (j + 1))
                                if qk == 0:
                                    nc.scalar.copy(qT[:, m, sl], pss[j][:])
                                elif qk == 2:
                                    nc.scalar.copy(vT[:, m, sl], pss[j][:])
                                else:
                                    # scatter psum head-halves into kTz rows
                                    nc.scalar.copy(kTz[0:64, 2 * m, sl],
                                                   pss[j][0:64, :])
                                    nc.scalar.copy(kTz[64:128, 2 * m + 1, sl],
                                                   pss[j][64:128, :])
                    # v natural via PE transpose of vT 128x128 blocks
                    for m in range(2):
                        for i in range(NT):
                            ps = psv.tile([128, 128], F32, tag="vtp", name=f"vtp{m}_{i}")
                            nc.tensor.transpose(
                                ps[:], vT[:, m, 128 * i:128 * (i + 1)].bitcast(F32),
                                ident[:])
                            nc.vector.tensor_copy(
                                vaug[:, i, 2 * m:2 * m + 2, 0:D],
                                ps[:].rearrange("p (h d) -> p h d", h=2))

                # -------- phase C+D: attention, interleaved with out-proj ----
                if "C" not in phases and "D" not in phases:
                    return
                with (
                    tc.tile_pool(name="cd", bufs=1) as pcd,
                    tc.tile_pool(name="pt", bufs=4) as ppt,
                    tc.tile_pool(name="small", bufs=4) as psm,
                    tc.tile_pool(name="ys", bufs=3) as pys,
                    tc.tile_pool(name="psst", bufs=2, space="PSUM") as psst,
                    tc.tile_pool(name="pspv", bufs=3, space="PSUM") as pspv,
                    tc.tile_pool(name="psy", bufs=1, space="PSUM") as psy,
                ):
                    if n_uniq > 0:
                        mks = pcd.tile([128, n_uniq, 512], F32)
                        nc.sync.dma_start(mks[:], mk_ap.rearrange("p (u f) -> p u f", f=512))
                    woT = pcd.tile([128, 2, C], F32R)
                    nc.sync.dma_start(woT[:], woT_ap.rearrange("p (k m) -> p k m", k=2))
                    attnoutT = pcd.tile([128, 2, T], F32R)

                    for j in range(NJ):
                        blocks = [(i, bi) for i, bi in enumerate(block_info[j])
                                  if bi is not None]
                        chunks = [blocks[c:c + 2] for c in range(0, len(blocks), 2)]
                        for h in range(HG if "C" in phases else 0):
                            m = h // 2
                            jsl = slice(512 * j, 512 * (j + 1))
                            pv = pspv.tile([128, 512], F32, tag="pv", name=f"pv{h}_{j}")
                            n_acc = len(blocks)
                            acc = 0
                            prev_chunk = None  # (pt, idxs)

                            def emit_pv(pt, idxs):
                                nonlocal acc
                                for c, i in enumerate(idxs):
                                    nc.tensor.matmul(
                                        pv[:], vaug[:, i, h, :], pt[:, c, :],
                                        start=(acc == 0), stop=(acc == n_acc - 1))
                                    acc += 1

                            for ch in chunks:
                                nsub = len(ch)
                                st = psst.tile([128, 2, 512], F32, tag="st", name=f"st{h}_{j}")
                                for c, (i, bi) in enumerate(ch):
                                    stair = (isinstance(bi, tuple)
                                             and bi[0] == "st")
                                    nc.tensor.matmul(
                                        st[:, c, :],
                                        kTz[:, h, 128 * i:128 * (i + 1)],
                                        qT[:, m, jsl],
                                        start=True, stop=not stair)
                                    if stair:
                                        # add -1e5 to causally-invalid slots
                                        nc.tensor.matmul(
                                            st[:, c, :], triA[:],
                                            bsh[:, bi[1], :],
                                            start=False, stop=True)
                                pt = ppt.tile([128, 2, 512], F32R, tag="pt")
                                # one exp per chunk: the ~0.9us fixed per-op
                                # ACT cost dominates, so amortize over 1024
                                nc.scalar.activation(
                                    pt[:, 0:nsub, :], st[:, 0:nsub, :],
                                    mybir.ActivationFunctionType.Exp, scale=SCALE)
                                mi = [bi for _, bi in ch]
                                for c, b in enumerate(mi):
                                    if isinstance(b, tuple) and b[0] == "mk":
                                        nc.vector.tensor_mul(
                                            pt[:, c, :], pt[:, c, :],
                                            mks[:, b[1], :])
                                if prev_chunk is not None:
                                    emit_pv(*prev_chunk)
                                prev_chunk = (pt, [i for i, _ in ch])
                            emit_pv(*prev_chunk)
                            # normalization: 1/denom (row 64) broadcast down
                            # 128 partitions via ones-column matmul; fast
                            # approx recip (18 bits) wants an SBUF operand
                            dnm = psm.tile([1, 512], F32, tag="dnm")
                            nc.vector.tensor_copy(dnm[:], pv[64:65, :])
                            recip = psm.tile([1, 512], F32, tag="recip")
                            nc.vector.reciprocal_approx_fast(recip[:], dnm[:])
                            nc.vector.tensor_copy(recipz[0:1, :], recip[:])
                            bc = psy.tile([128, 512], F32, tag="yps", name=f"bc{h}_{j}")
                            nc.tensor.matmul(bc[:], ones128[:], recipz[:],
                                             start=True, stop=True)
                            avu = psm.tile([64, 512], F32, tag="avu")
                            nc.vector.tensor_copy(avu[:], pv[0:64, :])
                            row = 64 * (h % 2)
                            nc.vector.tensor_mul(
                                attnoutT[row:row + 64, m, jsl],
                                avu[:], bc[0:64, :])

                        # ---- phase D for this j: y rows [512j, 512j+512) ----
                        if "D" not in phases:
                            continue
                        for tp in range(2):     # pairs of row tiles
                            ys = pys.tile([128, 2, C], F32, tag="ys")
                            for tsub in range(2):
                                t = 4 * j + 2 * tp + tsub
                                for o2 in range(2):
                                    ps = psy.tile([128, 512], F32, tag="yps", name=f"yps{t}_{o2}")
                                    for kc in range(2):
                                        nc.tensor.matmul(
                                            ps[:],
                                            attnoutT[:, kc, 128 * t:128 * (t + 1)],
                                            woT[:, kc, 512 * o2:512 * (o2 + 1)],
                                            start=(kc == 0), stop=(kc == 1))
                                    nc.vector.tensor_copy(
                                        ys[:, tsub, 512 * o2:512 * (o2 + 1)], ps[:])
                            r0 = 512 * j + 256 * tp
                            nc.scalar.dma_start(
                                y_ap[r0:r0 + 256, :].rearrange("(tt p) o -> p tt o", p=128),
                                ys[:])

            if loop_n is None:
                body()
            else:
                with tc.For_i(0, loop_n, 1):
                    body()

    nc.compile()
    return nc


# ---------------------------------------------------------------- run harness

def _install_verbose_hook():
    install_neuronx_cc_hook()
    try:
        import libneuronxla
    except ImportError:
        return
    import traceback
    inner = bass2jax.neuronx_cc_hook

    def wrapped(*a, **kw):
        try:
            return inner(*a, **kw)
        except BaseException:
            traceback.print_exc()
            raise
    libneuronxla.neuronx_cc = wrapped


class _SpmdRunner:
    def __init__(self, nc, n_cores):
        _install_verbose_hook()
        self.nc, self.n_cores = nc, n_cores
        pname = nc.partition_id_tensor.name if nc.partition_id_tensor else None
        in_names, out_names, out_avals = [], [], []
        for alloc in nc.m.functions[0].allocations:
            if not isinstance(alloc, mybir.MemoryLocationSet):
                continue
            name = alloc.memorylocations[0].name
            if alloc.kind == "ExternalInput":
                if name != pname:
                    in_names.append(name)
            elif alloc.kind == "ExternalOutput":
                out_names.append(name)
                out_avals.append(jax.core.ShapedArray(
                    tuple(alloc.tensor_shape), mybir.dt.np(alloc.dtype)))
        self.in_names, self.out_names, self.out_avals = in_names, out_names, out_avals
        n_params = len(in_names)
        all_in = list(in_names) + list(out_names)
        if pname is not None:
            all_in.append(pname)

        def _body(*args):
            operands = list(args)
            if pname is not None:
                operands.append(partition_id_tensor())
            return tuple(_bass_exec_p.bind(
                *operands,
                out_avals=tuple(out_avals), in_names=tuple(all_in),
                out_names=tuple(out_names), lowering_input_output_aliases=(),
                sim_require_finite=True, sim_require_nnan=True, nc=nc))

        devices = jax.devices()[:n_cores]
        self.mesh = Mesh(np.asarray(devices), ("core",))
        in_specs = (PartitionSpec("core"),) * (n_params + len(out_names))
        out_specs = (PartitionSpec("core"),) * len(out_names)
        self.fn = jax.jit(shard_map(_body, mesh=self.mesh, in_specs=in_specs,
                                    out_specs=out_specs, check_rep=False),
                          keep_unused=True)
        self._shard = jax.sharding.NamedSharding(self.mesh, PartitionSpec("core"))

    def put_inputs(self, in_maps):
        arrs = []
        for name in self.in_names:
            cat = np.concatenate([np.asarray(m[name]) for m in in_maps], axis=0)
            arrs.append(jax.device_put(cat, self._shard))
        for av in self.out_avals:
            z = np.zeros((self.n_cores * av.shape[0], *av.shape[1:]), av.dtype)
            arrs.append(jax.device_put(z, self._shard))
        return arrs

    def run(self, dev_args):
        outs = self.fn(*dev_args)
        jax.block_until_ready(outs)
        return outs

    def results(self, outs):
        per_core = []
        for c in range(self.n_cores):
            per_core.append({
                name: np.asarray(outs[i]).reshape(
                    self.n_cores, *self.out_avals[i].shape)[c]
                for i, name in enumerate(self.out_names)})
        return per_core


# ---------------------------------------------------------------- host side

def _mask_blocks(mask):
    """Classify transposed 128x512 blocks of the [T,T] mask.

    Returns (block_info, uniq, shifts) where block_info[j][i] is None (all
    masked), -1 (all valid), ("st", slot) (causal staircase valid = p <=
    f - shifts[slot], applied on-device as a matmul accumulation), or
    ("mk", idx) (arbitrary mixed pattern, multiplied from uniq[idx]).
    """
    m2 = np.asarray(mask).reshape(T, T)
    valid = (m2 != -np.inf)          # [tq, tk]
    validT = valid.T                 # [tk, tq]
    uniq, keys = [], {}
    shifts, shift_keys = [], {}
    p_idx = np.arange(128)[:, None]
    f_idx = np.arange(512)[None, :]
    block_info = []
    for j in range(NJ):
        row = []
        for i in range(NT):
            blk = validT[128 * i:128 * (i + 1), 512 * j:512 * (j + 1)]
            if not blk.any():
                row.append(None)
                continue
            if blk.all():
                row.append(-1)
                continue
            s = 128 * i - 512 * j
            if -512 < s < 512 and np.array_equal(blk, p_idx <= f_idx - s):
                if s not in shift_keys:
                    shift_keys[s] = len(shifts)
                    shifts.append(s)
                row.append(("st", shift_keys[s]))
                continue
            k = hashlib.sha1(np.ascontiguousarray(blk)).hexdigest()
            if k not in keys:
                keys[k] = len(uniq)
                uniq.append(blk.astype(np.float32))
            row.append(("mk", keys[k]))
        block_info.append(row)
    return block_info, uniq, shifts


_MASK_BIG = -1.0e5


def _stair_operands(shifts):
    """triA [128,128] and bsh [128, n_shift, 512] with
    (triA.T @ bsh[:, slot])[p, f] = -1e5 where p > f - shifts[slot] else 0."""
    k_i = np.arange(128)
    p_i = np.arange(128)
    triA = ((p_i[None, :] > k_i[:, None]) | (k_i[:, None] == 127)
            ).astype(np.float32)
    bsh = np.zeros((128, len(shifts), 512), np.float32)
    for slot, s in enumerate(shifts):
        for f in range(512):
            g = f - s
            if g < 0:
                bsh[127, slot, f] = _MASK_BIG
            elif g <= 126:
                bsh[g, slot, f] = _MASK_BIG
    return triA, bsh


_CACHE = {}


def _get_runner(block_info, n_uniq, loop_n=None, phases="BCD", cast_dma=True):
    key = (str(block_info), n_uniq, loop_n, phases, cast_dma)
    if key not in _CACHE:
        nc = _build_nc(block_info, n_uniq, loop_n=loop_n, phases=phases, cast_dma=cast_dma)
        _CACHE[key] = _SpmdRunner(nc, N_CORES)
    return _CACHE[key]


def _round_f32r(a):
    """Round fp32 to f32r (RNE to 11 mantissa bits) so device DMA loads are
    PE-ready without a casting pass."""
    x = np.ascontiguousarray(a, np.float32).view(np.uint32).astype(np.uint64)
    half = np.uint64(0x7FF)
    out = (x + half + ((x >> np.uint64(12)) & np.uint64(1))) & np.uint64(0xFFFFF000)
    return out.astype(np.uint32).view(np.float32)


def _pack_rows(a):
    """[R*128, F] -> [128, R*F]: partition-contiguous packing for fast DMA."""
    r = a.shape[0] // 128
    return np.ascontiguousarray(
        a.reshape(r, 128, a.shape[1]).transpose(1, 0, 2).reshape(128, -1))


def _make_in_maps(x, mask, wq, wk, wv, wo):
    block_info, uniq, shifts = _mask_blocks(mask)
    x = np.asarray(x, np.float32)
    extra = {}
    if uniq:
        mk = np.stack(uniq)    # [u,128,512] -> [128, u*512]
        extra["mk"] = np.ascontiguousarray(
            mk.transpose(1, 0, 2).reshape(128, -1))
    if shifts:
        triA, bsh = _stair_operands(shifts)
        extra["triA"] = triA
        extra["bsh"] = np.ascontiguousarray(bsh.reshape(128, -1))
    in_maps = []
    for c in range(N_CORES):
        b, g = c // 4, c % 4
        sl = slice(HC * g, HC * (g + 1))
        in_maps.append({
            "xT": _round_f32r(x[b].T),
            "wqT": _pack_rows(_round_f32r(np.asarray(wq)[sl, :].T)),
            "wkT": _pack_rows(_round_f32r(np.asarray(wk)[sl, :].T)),
            "wvT": _pack_rows(_round_f32r(np.asarray(wv)[sl, :].T)),
            "woT": _pack_rows(_round_f32r(np.asarray(wo)[:, sl].T)),
            **extra,
        })
    return in_maps, block_info, len(uniq)


def kernel(x, mask, wq, wk, wv, wo):
    in_maps, block_info, n_uniq = _make_in_maps(x, mask, wq, wk, wv, wo)
    runner = _get_runner(block_info, n_uniq)
    dev = runner.put_inputs(in_maps)
    res = runner.results(runner.run(dev))
    out = np.zeros((B, T, C), np.float32)
    for c in range(N_CORES):
        out[c // 4] += res[c]["y"]
    return out



# revision 17
# speedup vs baseline: 1.3154x; 1.1945x over previous
"""Causal multi-head attention (B=2, T=2048, C=1024, H=16, D=64) on 8 trn2 cores.

Sharding: core c -> (batch c//4, head-group c%4 of 4 heads / 256 channels).
Each core computes q/k/v for its head group, causal attention, and a partial
output projection y_part[2048,1024] = attnout_g @ wo_g.T. The host sums the 4
per-group partials of each batch (the "all-reduce after wo" done host-side).

Device kernel (per core, SPMD identical program), bf16 datapath (PE rate is
the same as f32r; bf16 halves SBUF/DMA and enables FWL weight loads):
  phase B: xT,[wq|wk|wv]T loaded as bf16; q^T,k^T (head-channel major, bf16)
           and v (natural, with a ones column -> vaug) via PE matmuls.
  phase C: per head, per tq-tile(512): ST[tk128,tq512] = k^T.T @ q^T; causal
           staircase mask applied INSIDE the score psum group as an extra
           matmul (triA.T @ bsh adds -98304 to invalid slots); P = exp(ST/8)
           on ScalarE (PSUM->SBUF bf16); PV accumulated over tk into
           PSUM[65,512] where row 64 (ones column of vaug) is the softmax
           denominator; normalize with approx-reciprocal + K=1 bcast matmul.
  phase D: y[t,1024] = attnoutT.T @ woT, DMA'd out (bf16) per 256-row tile.

All tile pools live OUTSIDE the timing loop so iteration n+1's input DMAs
overlap iteration n's attention (no SBUF-region reuse between phases).
"""
import time
import hashlib
import numpy as np
import ml_dtypes

import jax
import jax.numpy as jnp
from jax.sharding import Mesh, PartitionSpec
from jax.experimental.shard_map import shard_map

import concourse.bass as bass
import concourse.tile as tile
from concourse import bacc, mybir
from concourse import bass2jax
from concourse.bass2jax import _bass_exec_p, install_neuronx_cc_hook, partition_id_tensor

B, T, C = 2, 2048, 1024
H = 16
D = C // H            # 64
SCALE = D ** -0.5     # 0.125
N_CORES = 8
HG = H // (N_CORES // B)   # heads per core = 4
HC = HG * D                # channels per core = 256
KT = C // 128              # 8 contraction tiles
NT = T // 128              # 16 row tiles
NJ = T // 512              # 4 tq tiles
F32 = mybir.dt.float32
F32R = mybir.dt.float32r
BF16 = mybir.dt.bfloat16


# ---------------------------------------------------------------- device code

def _build_nc(block_info, n_uniq, loop_n=None, phases="BCD", cast_dma=False):
    """block_info[j][i] = None (skip) | -1 (full) | ("st", slot) (causal
    staircase applied as a PE matmul accumulation) | ("mk", idx>=0) (general
    mask tile multiplied on DVE)."""
    n_shift = 1 + max([e[1] for row in block_info for e in row
                       if isinstance(e, tuple) and e[0] == "st"], default=-1)
    nc = bacc.Bacc("TRN2", target_bir_lowering=False, debug=False,
                   num_devices=N_CORES)
    xT_ap = nc.dram_tensor("xT", [C, T], BF16, kind="ExternalInput").ap()
    # weights host-packed so each SBUF partition's bytes are contiguous in
    # DRAM (large descriptors; scattered 1KB descriptors measured ~4x slower)
    wqT_ap = nc.dram_tensor("wqT", [128, KT * HC], BF16, kind="ExternalInput").ap()
    wkT_ap = nc.dram_tensor("wkT", [128, KT * HC], BF16, kind="ExternalInput").ap()
    wvT_ap = nc.dram_tensor("wvT", [128, KT * HC], BF16, kind="ExternalInput").ap()
    woT_ap = nc.dram_tensor("woT", [128, 2 * C], BF16, kind="ExternalInput").ap()
    if n_uniq > 0:
        mk_ap = nc.dram_tensor("mk", [128, n_uniq * 512], F32,
                               kind="ExternalInput").ap()
    if n_shift > 0:
        triA_ap = nc.dram_tensor("triA", [128, 128], BF16,
                                 kind="ExternalInput").ap()
        bsh_ap = nc.dram_tensor("bsh", [128, n_shift * 512], BF16,
                                kind="ExternalInput").ap()
    y_ap = nc.dram_tensor("y", [T, C], BF16, kind="ExternalOutput").ap()

    with tile.TileContext(nc) as tc:
        with (
            nc.allow_low_precision(reason="bf16 matmul pipeline"),
            tc.tile_pool(name="glob", bufs=1) as pg,
            tc.tile_pool(name="warm", bufs=1) as pwarm,
            tc.tile_pool(name="ab", bufs=1) as pab,
            tc.tile_pool(name="wp", bufs=2) as pwp,
            tc.tile_pool(name="cd", bufs=1) as pcd,
            tc.tile_pool(name="pt", bufs=4) as ppt,
            tc.tile_pool(name="small", bufs=4) as psm,
            tc.tile_pool(name="ys", bufs=3) as pys,
            # PSUM: 2x [128,2,512] (4 banks) + 4x [128,512] (4 banks)
            tc.tile_pool(name="pst", bufs=2, space="PSUM") as pst,
            tc.tile_pool(name="ppv", bufs=4, space="PSUM") as ppv,
        ):
            # persistent across phases
            qT = pg.tile([128, 2, T], BF16)        # [o-part, o-tile, t]
            # k^T zero-padded per head: head h lives in partition rows
            # 64*(h%2)..+64 of kTz[:, h, :], other rows stay 0 so the score
            # matmul contracts K=128 against the 2-head-packed qT.
            kTz = pg.tile([128, HG, T], BF16)
            # v natural per tk-tile/head, padded to 128 cols: [v | 1 | zeros]
            # (128 cols keeps FWL weight loads; ones col gives softmax denom)
            vaug = pg.tile([128, NT, HG, 128], BF16)
            ones128 = pg.tile([128, 128], BF16)    # all-ones lhsT for bcast
            recipz = pg.tile([128, 512], BF16)     # row0=recip, rows1-127 zero
            ident = pg.tile([128, 128], F32)       # PE-transpose identity
            if n_shift > 0:
                triA = pg.tile([128, 128], BF16)
                bsh = pg.tile([128, n_shift, 512], BF16)
                nc.sync.dma_start(triA[:], triA_ap)
                nc.sync.dma_start(
                    bsh[:], bsh_ap.rearrange("p (s f) -> p s f", f=512))

            # warm the Exp table while DMAs run; zeros/ones are produced via
            # casting copies from f32 tiles (works for every dtype).
            wtile = pwarm.tile([1, 16], F32)
            nc.vector.memset(wtile[:], 0.0)
            nc.scalar.activation(wtile[:], wtile[:],
                                 mybir.ActivationFunctionType.Exp)
            zt = pwarm.tile([128, 512], F32)
            nc.vector.memset(zt[:], 0.0)
            nc.vector.tensor_copy(kTz[:].rearrange("p h (j f) -> p h j f", f=512), zt[:, None, None, :].broadcast_to([128, HG, NJ, 512]))
            nc.vector.tensor_copy(vaug[:], zt[:, None, None, 0:128].broadcast_to([128, NT, HG, 128]))
            nc.vector.tensor_copy(recipz[:], zt[:])
            import concourse.masks as _masks
            _masks.make_identity(nc, ident[:])
            ot = pwarm.tile([128, 16], F32)
            nc.vector.memset(ot[:], 1.0)
            nc.vector.tensor_copy(vaug[:, :, :, D], ot[:, None, 0:HG].broadcast_to([128, NT, HG]))
            nc.vector.tensor_copy(ones128[:], ot[:, 0:1].broadcast_to([128, 128]))

            # persistent phase tiles (hoisted so the loop can pipeline)
            xT = pab.tile([128, KT, T], BF16)
            vT = pab.tile([128, 2, T], F32R)
            woT = pcd.tile([128, 2, C], BF16)
            attnoutT = pcd.tile([128, 2, T], BF16)
            if n_uniq > 0:
                mks = pcd.tile([128, n_uniq, 512], F32)

            def body():
                # ---------------- phase A/B: loads + projections ------------
                # chunked so projections start as soon as chunk 0 lands
                for kc in range(KT):
                    nc.sync.dma_start(
                        xT[:, kc, :],
                        xT_ap.rearrange("(k p) t -> k p t", p=128)[kc])
                nc.sync.dma_start(woT[:], woT_ap.rearrange("p (k m) -> p k m", k=2))
                if n_uniq > 0:
                    nc.sync.dma_start(mks[:], mk_ap.rearrange("p (u f) -> p u f", f=512))
                if "B" not in phases:
                    return

                # q^T, k^T, v^T: [o, t] = w_g @ x^T; kc-inner per j-pair so
                # each psum pair drains while the next pair accumulates
                for w_ap, qk in ((wqT_ap, 0), (wkT_ap, 1), (wvT_ap, 2)):
                    w_t = pwp.tile([128, KT, HC], BF16, tag="w", name=f"w{qk}")
                    nc.sync.dma_start(w_t[:], w_ap.rearrange("p (k m) -> p k m", k=KT))
                    for m in range(2):
                        for jh in range(2):
                            pss = pst.tile([128, 2, 512], F32, tag="st",
                                           name=f"qkps{qk}_{m}_{jh}")
                            for kc in range(KT):
                                for j2 in range(2):
                                    j = 2 * jh + j2
                                    nc.tensor.matmul(
                                        pss[:, j2, :],
                                        w_t[:, kc, 128 * m:128 * (m + 1)],
                                        xT[:, kc, 512 * j:512 * (j + 1)],
                                        start=(kc == 0), stop=(kc == KT - 1))
                            for j2 in range(2):
                                j = 2 * jh + j2
                                sl = slice(512 * j, 512 * (j + 1))
                                if qk == 0:
                                    nc.scalar.copy(qT[:, m, sl], pss[:, j2, :])
                                elif qk == 2:
                                    nc.scalar.copy(vT[:, m, sl], pss[:, j2, :])
                                else:
                                    # scatter psum head-halves into kTz rows
                                    nc.scalar.copy(kTz[0:64, 2 * m, sl],
                                                   pss[0:64, j2, :])
                                    nc.scalar.copy(kTz[64:128, 2 * m + 1, sl],
                                                   pss[64:128, j2, :])
                # v natural via PE transpose of vT 128x128 blocks
                for m in range(2):
                    for i in range(NT):
                        ps = ppv.tile([128, 512], F32, tag="pv", name=f"vtp{m}_{i}")
                        nc.tensor.transpose(
                            ps[:, 0:128], vT[:, m, 128 * i:128 * (i + 1)].bitcast(F32),
                            ident[:])
                        nc.vector.tensor_copy(
                            vaug[:, i, 2 * m:2 * m + 2, 0:D],
                            ps[:, 0:128].rearrange("p (h d) -> p h d", h=2))

                # -------- phase C+D: attention, interleaved with out-proj ----
                if "C" not in phases and "D" not in phases:
                    return
                for j in range(NJ):
                    blocks = [(i, bi) for i, bi in enumerate(block_info[j])
                              if bi is not None]
                    chunks = [blocks[c:c + 2] for c in range(0, len(blocks), 2)]
                    for h in range(HG if "C" in phases else 0):
                        m = h // 2
                        jsl = slice(512 * j, 512 * (j + 1))
                        pv = ppv.tile([128, 512], F32, tag="pv", name=f"pv{h}_{j}")
                        n_acc = len(blocks)
                        acc = 0
                        prev_chunk = None  # (pt, idxs)

                        def emit_pv(pt, idxs):
                            nonlocal acc
                            for c, i in enumerate(idxs):
                                nc.tensor.matmul(
                                    pv[:], vaug[:, i, h, :], pt[:, c, :],
                                    start=(acc == 0), stop=(acc == n_acc - 1))
                                acc += 1

                        for ch in chunks:
                            nsub = len(ch)
                            st = pst.tile([128, 2, 512], F32, tag="st", name=f"st{h}_{j}")
                            for c, (i, bi) in enumerate(ch):
                                stair = (isinstance(bi, tuple)
                                         and bi[0] == "st")
                                nc.tensor.matmul(
                                    st[:, c, :],
                                    kTz[:, h, 128 * i:128 * (i + 1)],
                                    qT[:, m, jsl],
                                    start=True, stop=not stair)
                                if stair:
                                    # add -98304 to causally-invalid slots
                                    nc.tensor.matmul(
                                        st[:, c, :], triA[:],
                                        bsh[:, bi[1], :],
                                        start=False, stop=True)
                            pt = ppt.tile([128, 2, 512], BF16, tag="pt")
                            # one exp per chunk: the fixed per-op ACT cost
                            # (352 cyc) amortizes over 1024 elements
                            nc.scalar.activation(
                                pt[:, 0:nsub, :], st[:, 0:nsub, :],
                                mybir.ActivationFunctionType.Exp, scale=SCALE)
                            for c, (_, b) in enumerate(ch):
                                if isinstance(b, tuple) and b[0] == "mk":
                                    nc.vector.tensor_mul(
                                        pt[:, c, :], pt[:, c, :],
                                        mks[:, b[1], :])
                            if prev_chunk is not None:
                                emit_pv(*prev_chunk)
                            prev_chunk = (pt, [i for i, _ in ch])
                        emit_pv(*prev_chunk)
                        # normalization: 1/denom (row 64) broadcast down 128
                        # partitions via ones-column matmul; fast approx
                        # recip (18 bits) wants an SBUF f32 operand
                        dnm = psm.tile([1, 512], F32, tag="dnm")
                        nc.vector.tensor_copy(dnm[:], pv[64:65, :])
                        recip = psm.tile([1, 512], F32, tag="recip")
                        nc.vector.reciprocal_approx_fast(recip[:], dnm[:])
                        nc.vector.tensor_copy(recipz[0:1, :], recip[:])
                        bc = ppv.tile([128, 512], F32, tag="pv", name=f"bc{h}_{j}")
                        nc.tensor.matmul(bc[:], ones128[:], recipz[:],
                                         start=True, stop=True)
                        avu = psm.tile([64, 512], F32, tag="avu")
                        nc.vector.tensor_copy(avu[:], pv[0:64, :])
                        row = 64 * (h % 2)
                        nc.vector.tensor_mul(
                            attnoutT[row:row + 64, m, jsl],
                            avu[:], bc[0:64, :])

                    # ---- phase D for this j: y rows [512j, 512j+512) ----
                    if "D" not in phases:
                        continue
                    for tp in range(2):     # pairs of row tiles
                        ys = pys.tile([128, 2, C], BF16, tag="ys")
                        for tsub in range(2):
                            t = 4 * j + 2 * tp + tsub
                            for o2 in range(2):
                                ps = ppv.tile([128, 512], F32, tag="pv", name=f"yps{t}_{o2}")
                                for kc in range(2):
                                    nc.tensor.matmul(
                                        ps[:],
                                        attnoutT[:, kc, 128 * t:128 * (t + 1)],
                                        woT[:, kc, 512 * o2:512 * (o2 + 1)],
                                        start=(kc == 0), stop=(kc == 1))
                                nc.vector.tensor_copy(
                                    ys[:, tsub, 512 * o2:512 * (o2 + 1)], ps[:])
                        r0 = 512 * j + 256 * tp
                        nc.scalar.dma_start(
                            y_ap[r0:r0 + 256, :].rearrange("(tt p) o -> p tt o", p=128),
                            ys[:])

            if loop_n is None:
                body()
            else:
                with tc.For_i(0, loop_n, 1):
                    body()

    nc.compile()
    return nc


# ---------------------------------------------------------------- run harness

def _install_verbose_hook():
    install_neuronx_cc_hook()
    try:
        import libneuronxla
    except ImportError:
        return
    import traceback
    inner = bass2jax.neuronx_cc_hook

    def wrapped(*a, **kw):
        try:
            return inner(*a, **kw)
        except BaseException:
            traceback.print_exc()
            raise
    libneuronxla.neuronx_cc = wrapped


class _SpmdRunner:
    def __init__(self, nc, n_cores):
        _install_verbose_hook()
        self.nc, self.n_cores = nc, n_cores
        pname = nc.partition_id_tensor.name if nc.partition_id_tensor else None
        in_names, out_names, out_avals = [], [], []
        for alloc in nc.m.functions[0].allocations:
            if not isinstance(alloc, mybir.MemoryLocationSet):
                continue
            name = alloc.memorylocations[0].name
            if alloc.kind == "ExternalInput":
                if name != pname:
                    in_names.append(name)
            elif alloc.kind == "ExternalOutput":
                out_names.append(name)
                out_avals.append(jax.core.ShapedArray(
                    tuple(alloc.tensor_shape), mybir.dt.np(alloc.dtype)))
        self.in_names, self.out_names, self.out_avals = in_names, out_names, out_avals
        n_params = len(in_names)
        all_in = list(in_names) + list(out_names)
        if pname is not None:
            all_in.append(pname)

        def _body(*args):
            operands = list(args)
            if pname is not None:
                operands.append(partition_id_tensor())
            return tuple(_bass_exec_p.bind(
                *operands,
                out_avals=tuple(out_avals), in_names=tuple(all_in),
                out_names=tuple(out_names), lowering_input_output_aliases=(),
                sim_require_finite=True, sim_require_nnan=True, nc=nc))

        devices = jax.devices()[:n_cores]
        self.mesh = Mesh(np.asarray(devices), ("core",))
        in_specs = (PartitionSpec("core"),) * (n_params + len(out_names))
        out_specs = (PartitionSpec("core"),) * len(out_names)
        self.fn = jax.jit(shard_map(_body, mesh=self.mesh, in_specs=in_specs,
                                    out_specs=out_specs, check_rep=False),
                          keep_unused=True)
        self._shard = jax.sharding.NamedSharding(self.mesh, PartitionSpec("core"))

    def put_inputs(self, in_maps):
        arrs = []
        for name in self.in_names:
            cat = np.concatenate([np.asarray(m[name]) for m in in_maps], axis=0)
            arrs.append(jax.device_put(cat, self._shard))
        for av in self.out_avals:
            z = np.zeros((self.n_cores * av.shape[0], *av.shape[1:]), av.dtype)
            arrs.append(jax.device_put(z, self._shard))
        return arrs

    def run(self, dev_args):
        outs = self.fn(*dev_args)
        jax.block_until_ready(outs)
        return outs

    def results(self, outs):
        per_core = []
        for c in range(self.n_cores):
            per_core.append({
                name: np.asarray(outs[i]).reshape(
                    self.n_cores, *self.out_avals[i].shape)[c]
                for i, name in enumerate(self.out_names)})
        return per_core


# ---------------------------------------------------------------- host side

def _mask_blocks(mask):
    """Classify transposed 128x512 blocks of the [T,T] mask.

    Returns (block_info, uniq, shifts) where block_info[j][i] is None (all
    masked), -1 (all valid), ("st", slot) (causal staircase valid = p <=
    f - shifts[slot], applied on-device as a matmul accumulation), or
    ("mk", idx) (arbitrary mixed pattern, multiplied from uniq[idx])."""
    m2 = np.asarray(mask).reshape(T, T)
    valid = (m2 != -np.inf)          # [tq, tk]
    validT = valid.T                 # [tk, tq]
    uniq, keys = [], {}
    shifts, shift_keys = [], {}
    p_idx = np.arange(128)[:, None]
    f_idx = np.arange(512)[None, :]
    block_info = []
    for j in range(NJ):
        row = []
        for i in range(NT):
            blk = validT[128 * i:128 * (i + 1), 512 * j:512 * (j + 1)]
            if not blk.any():
                row.append(None)
                continue
            if blk.all():
                row.append(-1)
                continue
            s = 128 * i - 512 * j
            if -512 < s < 512 and np.array_equal(blk, p_idx <= f_idx - s):
                if s not in shift_keys:
                    shift_keys[s] = len(shifts)
                    shifts.append(s)
                row.append(("st", shift_keys[s]))
                continue
            k = hashlib.sha1(np.ascontiguousarray(blk)).hexdigest()
            if k not in keys:
                keys[k] = len(uniq)
                uniq.append(blk.astype(np.float32))
            row.append(("mk", keys[k]))
        block_info.append(row)
    return block_info, uniq, shifts


_MASK_BIG = -98304.0    # -1.5 * 2**16, exact in bf16


def _stair_operands(shifts):
    """triA [128,128] and bsh [128, n_shift, 512] with
    (triA.T @ bsh[:, slot])[p, f] = _MASK_BIG where p > f - shifts[slot]."""
    k_i = np.arange(128)
    p_i = np.arange(128)
    triA = ((p_i[None, :] > k_i[:, None]) | (k_i[:, None] == 127)
            ).astype(np.float32)
    bsh = np.zeros((128, len(shifts), 512), np.float32)
    for slot, s in enumerate(shifts):
        for f in range(512):
            g = f - s
            if g < 0:
                bsh[127, slot, f] = _MASK_BIG
            elif g <= 126:
                bsh[g, slot, f] = _MASK_BIG
    return triA, bsh


_CACHE = {}


def _get_runner(block_info, n_uniq, loop_n=None, phases="BCD", cast_dma=True):
    key = (str(block_info), n_uniq, loop_n, phases, cast_dma)
    if key not in _CACHE:
        nc = _build_nc(block_info, n_uniq, loop_n=loop_n, phases=phases, cast_dma=cast_dma)
        _CACHE[key] = _SpmdRunner(nc, N_CORES)
    return _CACHE[key]


def _bf16(a):
    return np.ascontiguousarray(np.asarray(a, np.float32)).astype(
        ml_dtypes.bfloat16)


def _pack_rows(a):
    """[R*128, F] -> [128, R*F]: partition-contiguous packing for fast DMA."""
    r = a.shape[0] // 128
    return np.ascontiguousarray(
        a.reshape(r, 128, a.shape[1]).transpose(1, 0, 2).reshape(128, -1))


def _make_in_maps(x, mask, wq, wk, wv, wo):
    block_info, uniq, shifts = _mask_blocks(mask)
    x = np.asarray(x, np.float32)
    extra = {}
    if uniq:
        mk = np.stack(uniq)    # [u,128,512] -> [128, u*512]
        extra["mk"] = np.ascontiguousarray(
            mk.transpose(1, 0, 2).reshape(128, -1))
    if shifts:
        triA, bsh = _stair_operands(shifts)
        extra["triA"] = _bf16(triA)
        extra["bsh"] = _bf16(np.ascontiguousarray(bsh.reshape(128, -1)))
    in_maps = []
    for c in range(N_CORES):
        b, g = c // 4, c % 4
        sl = slice(HC * g, HC * (g + 1))
        in_maps.append({
            "xT": _bf16(x[b].T),
            "wqT": _pack_rows(_bf16(np.asarray(wq)[sl, :].T)),
            "wkT": _pack_rows(_bf16(np.asarray(wk)[sl, :].T)),
            "wvT": _pack_rows(_bf16(np.asarray(wv)[sl, :].T)),
            "woT": _pack_rows(_bf16(np.asarray(wo)[:, sl].T)),
            **extra,
        })
    return in_maps, block_info, len(uniq)


def kernel(x, mask, wq, wk, wv, wo):
    in_maps, block_info, n_uniq = _make_in_maps(x, mask, wq, wk, wv, wo)
    runner = _get_runner(block_info, n_uniq)
    dev = runner.put_inputs(in_maps)
    res = runner.results(runner.run(dev))
    out = np.zeros((B, T, C), np.float32)
    for c in range(N_CORES):
        out[c // 4] += res[c]["y"].astype(np.float32)
    return out


# revision 18
# speedup vs baseline: 1.3384x; 1.0174x over previous
"""Causal multi-head attention (B=2, T=2048, C=1024, H=16, D=64) on 8 trn2 cores.

Sharding: core c -> (batch c//4, head-group c%4 of 4 heads / 256 channels).
Each core computes q/k/v for its head group, causal attention, and a partial
output projection y_part[2048,1024] = attnout_g @ wo_g.T. The host sums the 4
per-group partials of each batch (the "all-reduce after wo" done host-side).

Device kernel (per core, SPMD identical program), bf16 datapath (PE rate is
the same as f32r; bf16 halves SBUF/DMA and enables FWL weight loads):
  phase B: xT,[wq|wk|wv]T loaded as bf16; q^T,k^T (head-channel major, bf16)
           and v (natural, with a ones column -> vaug) via PE matmuls.
  phase C: per head, per tq-tile(512): ST[tk128,tq512] = k^T.T @ q^T; causal
           staircase mask applied INSIDE the score psum group as an extra
           matmul (triA.T @ bsh adds -98304 to invalid slots); P = exp(ST/8)
           on ScalarE (PSUM->SBUF bf16); PV accumulated over tk into
           PSUM[65,512] where row 64 (ones column of vaug) is the softmax
           denominator; normalize with approx-reciprocal + K=1 bcast matmul.
  phase D: y[t,1024] = attnoutT.T @ woT, DMA'd out (bf16) per 256-row tile.

All tile pools live OUTSIDE the timing loop so iteration n+1's input DMAs
overlap iteration n's attention (no SBUF-region reuse between phases).
"""
import time
import hashlib
import numpy as np
import ml_dtypes

import jax
import jax.numpy as jnp
from jax.sharding import Mesh, PartitionSpec
from jax.experimental.shard_map import shard_map

import concourse.bass as bass
import concourse.tile as tile
from concourse import bacc, mybir
from concourse import bass2jax
from concourse.bass2jax import _bass_exec_p, install_neuronx_cc_hook, partition_id_tensor

B, T, C = 2, 2048, 1024
H = 16
D = C // H            # 64
SCALE = D ** -0.5     # 0.125
N_CORES = 8
HG = H // (N_CORES // B)   # heads per core = 4
HC = HG * D                # channels per core = 256
KT = C // 128              # 8 contraction tiles
NT = T // 128              # 16 row tiles
NJ = T // 512              # 4 tq tiles
F32 = mybir.dt.float32
F32R = mybir.dt.float32r
BF16 = mybir.dt.bfloat16


# ---------------------------------------------------------------- device code

def _build_nc(block_info, n_uniq, loop_n=None, phases="BCD", cast_dma=False):
    """block_info[j][i] = None (skip) | -1 (full) | ("st", slot) (causal
    staircase applied as a PE matmul accumulation) | ("mk", idx>=0) (general
    mask tile multiplied on DVE)."""
    n_shift = 1 + max([e[1] for row in block_info for e in row
                       if isinstance(e, tuple) and e[0] == "st"], default=-1)
    nc = bacc.Bacc("TRN2", target_bir_lowering=False, debug=False,
                   num_devices=N_CORES)
    xT_ap = nc.dram_tensor("xT", [C, T], BF16, kind="ExternalInput").ap()
    # weights host-packed so each SBUF partition's bytes are contiguous in
    # DRAM (large descriptors; scattered 1KB descriptors measured ~4x slower)
    wqT_ap = nc.dram_tensor("wqT", [128, KT * HC], BF16, kind="ExternalInput").ap()
    wkT_ap = nc.dram_tensor("wkT", [128, KT * HC], BF16, kind="ExternalInput").ap()
    wvT_ap = nc.dram_tensor("wvT", [128, KT * HC], BF16, kind="ExternalInput").ap()
    woT_ap = nc.dram_tensor("woT", [128, 2 * C], BF16, kind="ExternalInput").ap()
    if n_uniq > 0:
        mk_ap = nc.dram_tensor("mk", [128, n_uniq * 512], F32,
                               kind="ExternalInput").ap()
    if n_shift > 0:
        triA_ap = nc.dram_tensor("triA", [128, 128], BF16,
                                 kind="ExternalInput").ap()
        bsh_ap = nc.dram_tensor("bsh", [128, n_shift * 512], BF16,
                                kind="ExternalInput").ap()
    y_ap = nc.dram_tensor("y", [T, C], BF16, kind="ExternalOutput").ap()

    with tile.TileContext(nc) as tc:
        with (
            nc.allow_low_precision(reason="bf16 matmul pipeline"),
            tc.tile_pool(name="glob", bufs=1) as pg,
            tc.tile_pool(name="warm", bufs=1) as pwarm,
            tc.tile_pool(name="ab", bufs=1) as pab,
            tc.tile_pool(name="wp", bufs=2) as pwp,
            tc.tile_pool(name="cd", bufs=1) as pcd,
            tc.tile_pool(name="pt", bufs=4) as ppt,
            tc.tile_pool(name="small", bufs=4) as psm,
            tc.tile_pool(name="ys", bufs=3) as pys,
            # PSUM: 2x [128,2,512] (4 banks) + 4x [128,512] (4 banks)
            tc.tile_pool(name="pst", bufs=2, space="PSUM") as pst,
            tc.tile_pool(name="ppv", bufs=4, space="PSUM") as ppv,
        ):
            # persistent across phases
            qT = pg.tile([128, 2, T], BF16)        # [o-part, o-tile, t]
            # k^T zero-padded per head: head h lives in partition rows
            # 64*(h%2)..+64 of kTz[:, h, :], other rows stay 0 so the score
            # matmul contracts K=128 against the 2-head-packed qT.
            kTz = pg.tile([128, HG, T], BF16)
            # v natural per tk-tile/head, padded to 128 cols: [v | 1 | zeros]
            # (128 cols keeps FWL weight loads; ones col gives softmax denom)
            vaug = pg.tile([128, NT, HG, 128], BF16)
            ones128 = pg.tile([128, 128], BF16)    # all-ones lhsT for bcast
            recipz = pg.tile([128, 512], BF16)     # row0=recip, rows1-127 zero
            ident = pg.tile([128, 128], F32)       # PE-transpose identity
            if n_shift > 0:
                triA = pg.tile([128, 128], BF16)
                bsh = pg.tile([128, n_shift, 512], BF16)
                nc.sync.dma_start(triA[:], triA_ap)
                nc.sync.dma_start(
                    bsh[:], bsh_ap.rearrange("p (s f) -> p s f", f=512))

            # warm the Exp table while DMAs run; zeros/ones are produced via
            # casting copies from f32 tiles (works for every dtype).
            wtile = pwarm.tile([1, 16], F32)
            nc.vector.memset(wtile[:], 0.0)
            nc.scalar.activation(wtile[:], wtile[:],
                                 mybir.ActivationFunctionType.Exp)
            zt = pwarm.tile([128, 512], F32)
            nc.vector.memset(zt[:], 0.0)
            nc.vector.tensor_copy(kTz[:].rearrange("p h (j f) -> p h j f", f=512), zt[:, None, None, :].broadcast_to([128, HG, NJ, 512]))
            nc.vector.tensor_copy(vaug[:], zt[:, None, None, 0:128].broadcast_to([128, NT, HG, 128]))
            nc.vector.tensor_copy(recipz[:], zt[:])
            import concourse.masks as _masks
            _masks.make_identity(nc, ident[:])
            ot = pwarm.tile([128, 16], F32)
            nc.vector.memset(ot[:], 1.0)
            nc.vector.tensor_copy(vaug[:, :, :, D], ot[:, None, 0:HG].broadcast_to([128, NT, HG]))
            nc.vector.tensor_copy(ones128[:], ot[:, 0:1].broadcast_to([128, 128]))

            # persistent phase tiles (hoisted so the loop can pipeline)
            xT = pab.tile([128, KT, T], BF16)
            vT = pab.tile([128, 2, T], F32R)
            woT = pcd.tile([128, 2, C], BF16)
            attnoutT = pcd.tile([128, 2, T], BF16)
            if n_uniq > 0:
                mks = pcd.tile([128, n_uniq, 512], F32)

            def body():
                # ---------------- phase A/B: loads + projections ------------
                # chunked so projections start as soon as chunk 0 lands
                for kc in range(KT):
                    nc.sync.dma_start(
                        xT[:, kc, :],
                        xT_ap.rearrange("(k p) t -> k p t", p=128)[kc])
                nc.sync.dma_start(woT[:], woT_ap.rearrange("p (k m) -> p k m", k=2))
                if n_uniq > 0:
                    nc.sync.dma_start(mks[:], mk_ap.rearrange("p (u f) -> p u f", f=512))
                if "B" not in phases:
                    return

                # q^T, k^T, v^T: [o, t] = w_g @ x^T; kc-inner per j-pair so
                # each psum pair drains while the next pair accumulates
                for w_ap, qk in ((wqT_ap, 0), (wkT_ap, 1), (wvT_ap, 2)):
                    w_t = pwp.tile([128, KT, HC], BF16, tag="w", name=f"w{qk}")
                    nc.sync.dma_start(w_t[:], w_ap.rearrange("p (k m) -> p k m", k=KT))
                    for m in range(2):
                        for jh in range(2):
                            pss = pst.tile([128, 2, 512], F32, tag="st",
                                           name=f"qkps{qk}_{m}_{jh}")
                            for kc in range(KT):
                                for j2 in range(2):
                                    j = 2 * jh + j2
                                    nc.tensor.matmul(
                                        pss[:, j2, :],
                                        w_t[:, kc, 128 * m:128 * (m + 1)],
                                        xT[:, kc, 512 * j:512 * (j + 1)],
                                        start=(kc == 0), stop=(kc == KT - 1))
                            for j2 in range(2):
                                j = 2 * jh + j2
                                sl = slice(512 * j, 512 * (j + 1))
                                if qk == 0:
                                    nc.scalar.copy(qT[:, m, sl], pss[:, j2, :])
                                elif qk == 2:
                                    nc.scalar.copy(vT[:, m, sl], pss[:, j2, :])
                                else:
                                    # scatter psum head-halves into kTz rows
                                    nc.scalar.copy(kTz[0:64, 2 * m, sl],
                                                   pss[0:64, j2, :])
                                    nc.scalar.copy(kTz[64:128, 2 * m + 1, sl],
                                                   pss[64:128, j2, :])
                # v natural via PE transpose of vT 128x128 blocks
                for m in range(2):
                    for i in range(NT):
                        ps = ppv.tile([128, 512], F32, tag="pv", name=f"vtp{m}_{i}")
                        nc.tensor.transpose(
                            ps[:, 0:128], vT[:, m, 128 * i:128 * (i + 1)].bitcast(F32),
                            ident[:])
                        nc.vector.tensor_copy(
                            vaug[:, i, 2 * m:2 * m + 2, 0:D],
                            ps[:, 0:128].rearrange("p (h d) -> p h d", h=2))

                # -------- phase C+D: attention, interleaved with out-proj ----
                if "C" not in phases and "D" not in phases:
                    return
                for j in range(NJ):
                    blocks = [(i, bi) for i, bi in enumerate(block_info[j])
                              if bi is not None]
                    chunks = [blocks[c:c + 2] for c in range(0, len(blocks), 2)]
                    for h in range(HG if "C" in phases else 0):
                        m = h // 2
                        jsl = slice(512 * j, 512 * (j + 1))
                        pv = ppv.tile([128, 512], F32, tag="pv", name=f"pv{h}_{j}")
                        n_acc = len(blocks)
                        acc = 0
                        prev_chunk = None  # (pt, idxs)

                        def emit_pv(pt, idxs):
                            nonlocal acc
                            for c, i in enumerate(idxs):
                                nc.tensor.matmul(
                                    pv[:], vaug[:, i, h, :], pt[:, c, :],
                                    start=(acc == 0), stop=(acc == n_acc - 1))
                                acc += 1

                        for ch in chunks:
                            nsub = len(ch)
                            st = pst.tile([128, 2, 512], F32, tag="st", name=f"st{h}_{j}")
                            for c, (i, bi) in enumerate(ch):
                                stair = (isinstance(bi, tuple)
                                         and bi[0] == "st")
                                nc.tensor.matmul(
                                    st[:, c, :],
                                    kTz[:, h, 128 * i:128 * (i + 1)],
                                    qT[:, m, jsl],
                                    start=True, stop=not stair)
                                if stair:
                                    # add -98304 to causally-invalid slots
                                    nc.tensor.matmul(
                                        st[:, c, :], triA[:],
                                        bsh[:, bi[1], :],
                                        start=False, stop=True)
                            pt = ppt.tile([128, 2, 512], BF16, tag="pt")
                            # one exp per chunk: the fixed per-op ACT cost
                            # (352 cyc) amortizes over 1024 elements
                            nc.scalar.activation(
                                pt[:, 0:nsub, :], st[:, 0:nsub, :],
                                mybir.ActivationFunctionType.Exp, scale=SCALE)
                            for c, (_, b) in enumerate(ch):
                                if isinstance(b, tuple) and b[0] == "mk":
                                    nc.vector.tensor_mul(
                                        pt[:, c, :], pt[:, c, :],
                                        mks[:, b[1], :])
                            if prev_chunk is not None:
                                emit_pv(*prev_chunk)
                            prev_chunk = (pt, [i for i, _ in ch])
                        emit_pv(*prev_chunk)
                        # normalization: 1/denom (row 64) broadcast down 128
                        # partitions via ones-column matmul; fast approx
                        # recip (18 bits) wants an SBUF f32 operand
                        dnm = psm.tile([1, 512], F32, tag="dnm")
                        nc.vector.tensor_copy(dnm[:], pv[64:65, :])
                        recip = psm.tile([1, 512], F32, tag="recip")
                        nc.vector.reciprocal_approx_fast(recip[:], dnm[:])
                        nc.vector.tensor_copy(recipz[0:1, :], recip[:])
                        bc = ppv.tile([128, 512], F32, tag="pv", name=f"bc{h}_{j}")
                        nc.tensor.matmul(bc[:], ones128[:], recipz[:],
                                         start=True, stop=True)
                        avu = psm.tile([64, 512], F32, tag="avu")
                        nc.vector.tensor_copy(avu[:], pv[0:64, :])
                        row = 64 * (h % 2)
                        nc.vector.tensor_mul(
                            attnoutT[row:row + 64, m, jsl],
                            avu[:], bc[0:64, :])

                    # ---- phase D for this j: y rows [512j, 512j+512) ----
                    if "D" not in phases:
                        continue
                    for tp in range(2):     # pairs of row tiles
                        ys = pys.tile([128, 2, C], BF16, tag="ys")
                        for tsub in range(2):
                            t = 4 * j + 2 * tp + tsub
                            for o2 in range(2):
                                ps = ppv.tile([128, 512], F32, tag="pv", name=f"yps{t}_{o2}")
                                for kc in range(2):
                                    nc.tensor.matmul(
                                        ps[:],
                                        attnoutT[:, kc, 128 * t:128 * (t + 1)],
                                        woT[:, kc, 512 * o2:512 * (o2 + 1)],
                                        start=(kc == 0), stop=(kc == 1))
                                nc.vector.tensor_copy(
                                    ys[:, tsub, 512 * o2:512 * (o2 + 1)], ps[:])
                        r0 = 512 * j + 256 * tp
                        nc.scalar.dma_start(
                            y_ap[r0:r0 + 256, :].rearrange("(tt p) o -> p tt o", p=128),
                            ys[:])

            if loop_n is None:
                body()
            else:
                with tc.For_i(0, loop_n, 1, staggered_reset=True):
                    body()

    nc.compile()
    return nc


# ---------------------------------------------------------------- run harness

def _install_verbose_hook():
    install_neuronx_cc_hook()
    try:
        import libneuronxla
    except ImportError:
        return
    import traceback
    inner = bass2jax.neuronx_cc_hook

    def wrapped(*a, **kw):
        try:
            return inner(*a, **kw)
        except BaseException:
            traceback.print_exc()
            raise
    libneuronxla.neuronx_cc = wrapped


class _SpmdRunner:
    def __init__(self, nc, n_cores):
        _install_verbose_hook()
        self.nc, self.n_cores = nc, n_cores
        pname = nc.partition_id_tensor.name if nc.partition_id_tensor else None
        in_names, out_names, out_avals = [], [], []
        for alloc in nc.m.functions[0].allocations:
            if not isinstance(alloc, mybir.MemoryLocationSet):
                continue
            name = alloc.memorylocations[0].name
            if alloc.kind == "ExternalInput":
                if name != pname:
                    in_names.append(name)
            elif alloc.kind == "ExternalOutput":
                out_names.append(name)
                out_avals.append(jax.core.ShapedArray(
                    tuple(alloc.tensor_shape), mybir.dt.np(alloc.dtype)))
        self.in_names, self.out_names, self.out_avals = in_names, out_names, out_avals
        n_params = len(in_names)
        all_in = list(in_names) + list(out_names)
        if pname is not None:
            all_in.append(pname)

        def _body(*args):
            operands = list(args)
            if pname is not None:
                operands.append(partition_id_tensor())
            return tuple(_bass_exec_p.bind(
                *operands,
                out_avals=tuple(out_avals), in_names=tuple(all_in),
                out_names=tuple(out_names), lowering_input_output_aliases=(),
                sim_require_finite=True, sim_require_nnan=True, nc=nc))

        devices = jax.devices()[:n_cores]
        self.mesh = Mesh(np.asarray(devices), ("core",))
        in_specs = (PartitionSpec("core"),) * (n_params + len(out_names))
        out_specs = (PartitionSpec("core"),) * len(out_names)
        self.fn = jax.jit(shard_map(_body, mesh=self.mesh, in_specs=in_specs,
                                    out_specs=out_specs, check_rep=False),
                          keep_unused=True)
        self._shard = jax.sharding.NamedSharding(self.mesh, PartitionSpec("core"))

    def put_inputs(self, in_maps):
        arrs = []
        for name in self.in_names:
            cat = np.concatenate([np.asarray(m[name]) for m in in_maps], axis=0)
            arrs.append(jax.device_put(cat, self._shard))
        for av in self.out_avals:
            z = np.zeros((self.n_cores * av.shape[0], *av.shape[1:]), av.dtype)
            arrs.append(jax.device_put(z, self._shard))
        return arrs

    def run(self, dev_args):
        outs = self.fn(*dev_args)
        jax.block_until_ready(outs)
        return outs

    def results(self, outs):
        per_core = []
        for c in range(self.n_cores):
            per_core.append({
                name: np.asarray(outs[i]).reshape(
                    self.n_cores, *self.out_avals[i].shape)[c]
                for i, name in enumerate(self.out_names)})
        return per_core


# ---------------------------------------------------------------- host side

def _mask_blocks(mask):
    """Classify transposed 128x512 blocks of the [T,T] mask.

    Returns (block_info, uniq, shifts) where block_info[j][i] is None (all
    masked), -1 (all valid), ("st", slot) (causal staircase valid = p <=
    f - shifts[slot], applied on-device as a matmul accumulation), or
    ("mk", idx) (arbitrary mixed pattern, multiplied from uniq[idx])."""
    m2 = np.asarray(mask).reshape(T, T)
    valid = (m2 != -np.inf)          # [tq, tk]
    validT = valid.T                 # [tk, tq]
    uniq, keys = [], {}
    shifts, shift_keys = [], {}
    p_idx = np.arange(128)[:, None]
    f_idx = np.arange(512)[None, :]
    block_info = []
    for j in range(NJ):
        row = []
        for i in range(NT):
            blk = validT[128 * i:128 * (i + 1), 512 * j:512 * (j + 1)]
            if not blk.any():
                row.append(None)
                continue
            if blk.all():
                row.append(-1)
                continue
            s = 128 * i - 512 * j
            if -512 < s < 512 and np.array_equal(blk, p_idx <= f_idx - s):
                if s not in shift_keys:
                    shift_keys[s] = len(shifts)
                    shifts.append(s)
                row.append(("st", shift_keys[s]))
                continue
            k = hashlib.sha1(np.ascontiguousarray(blk)).hexdigest()
            if k not in keys:
                keys[k] = len(uniq)
                uniq.append(blk.astype(np.float32))
            row.append(("mk", keys[k]))
        block_info.append(row)
    return block_info, uniq, shifts


_MASK_BIG = -98304.0    # -1.5 * 2**16, exact in bf16


def _stair_operands(shifts):
    """triA [128,128] and bsh [128, n_shift, 512] with
    (triA.T @ bsh[:, slot])[p, f] = _MASK_BIG where p > f - shifts[slot]."""
    k_i = np.arange(128)
    p_i = np.arange(128)
    triA = ((p_i[None, :] > k_i[:, None]) | (k_i[:, None] == 127)
            ).astype(np.float32)
    bsh = np.zeros((128, len(shifts), 512), np.float32)
    for slot, s in enumerate(shifts):
        for f in range(512):
            g = f - s
            if g < 0:
                bsh[127, slot, f] = _MASK_BIG
            elif g <= 126:
                bsh[g, slot, f] = _MASK_BIG
    return triA, bsh


_CACHE = {}


def _get_runner(block_info, n_uniq, loop_n=None, phases="BCD", cast_dma=True):
    key = (str(block_info), n_uniq, loop_n, phases, cast_dma)
    if key not in _CACHE:
        nc = _build_nc(block_info, n_uniq, loop_n=loop_n, phases=phases, cast_dma=cast_dma)
        _CACHE[key] = _SpmdRunner(nc, N_CORES)
    return _CACHE[key]


def _bf16(a):
    return np.ascontiguousarray(np.asarray(a, np.float32)).astype(
        ml_dtypes.bfloat16)


def _pack_rows(a):
    """[R*128, F] -> [128, R*F]: partition-contiguous packing for fast DMA."""
    r = a.shape[0] // 128
    return np.ascontiguousarray(
        a.reshape(r, 128, a.shape[1]).transpose(1, 0, 2).reshape(128, -1))


def _make_in_maps(x, mask, wq, wk, wv, wo):
    block_info, uniq, shifts = _mask_blocks(mask)
    x = np.asarray(x, np.float32)
    extra = {}
    if uniq:
        mk = np.stack(uniq)    # [u,128,512] -> [128, u*512]
        extra["mk"] = np.ascontiguousarray(
            mk.transpose(1, 0, 2).reshape(128, -1))
    if shifts:
        triA, bsh = _stair_operands(shifts)
        extra["triA"] = _bf16(triA)
        extra["bsh"] = _bf16(np.ascontiguousarray(bsh.reshape(128, -1)))
    in_maps = []
    for c in range(N_CORES):
        b, g = c // 4, c % 4
        sl = slice(HC * g, HC * (g + 1))
        in_maps.append({
            "xT": _bf16(x[b].T),
            "wqT": _pack_rows(_bf16(np.asarray(wq)[sl, :].T)),
            "wkT": _pack_rows(_bf16(np.asarray(wk)[sl, :].T)),
            "wvT": _pack_rows(_bf16(np.asarray(wv)[sl, :].T)),
            "woT": _pack_rows(_bf16(np.asarray(wo)[:, sl].T)),
            **extra,
        })
    return in_maps, block_info, len(uniq)


def kernel(x, mask, wq, wk, wv, wo):
    in_maps, block_info, n_uniq = _make_in_maps(x, mask, wq, wk, wv, wo)
    runner = _get_runner(block_info, n_uniq)
    dev = runner.put_inputs(in_maps)
    res = runner.results(runner.run(dev))
    out = np.zeros((B, T, C), np.float32)
    for c in range(N_CORES):
        out[c // 4] += res[c]["y"].astype(np.float32)
    return out


# revision 19
# speedup vs baseline: 1.4610x; 1.0916x over previous
"""Causal multi-head attention (B=2, T=2048, C=1024, H=16, D=64) on 8 trn2 cores.

Sharding: core c -> (batch c//4, head-group c%4 of 4 heads / 256 channels).
Each core computes q/k/v for its head group, causal attention, and a partial
output projection y_part[2048,1024] = attnout_g @ wo_g.T. The host sums the 4
per-group partials of each batch (the "all-reduce after wo" done host-side).

Device kernel (per core, SPMD identical program), bf16 datapath (PE rate is
the same as f32r; bf16 halves SBUF/DMA and enables FWL weight loads):
  phase B: xT,[wq|wk|wv]T loaded as bf16; q^T,k^T (head-channel major, bf16)
           and v (natural, with a ones column -> vaug) via PE matmuls.
  phase C: per head, per tq-tile(512): ST[tk128,tq512] = k^T.T @ q^T; causal
           staircase mask applied INSIDE the score psum group as an extra
           matmul (triA.T @ bsh adds -98304 to invalid slots); P = exp(ST/8)
           on ScalarE (PSUM->SBUF bf16); PV accumulated over tk into
           PSUM[65,512] where row 64 (ones column of vaug) is the softmax
           denominator; normalize with approx-reciprocal + K=1 bcast matmul.
  phase D: y[t,1024] = attnoutT.T @ woT, DMA'd out (bf16) per 256-row tile.

All tile pools live OUTSIDE the timing loop so iteration n+1's input DMAs
overlap iteration n's attention (no SBUF-region reuse between phases).
"""
import time
import hashlib
import numpy as np
import ml_dtypes

import jax
import jax.numpy as jnp
from jax.sharding import Mesh, PartitionSpec
from jax.experimental.shard_map import shard_map

import concourse.bass as bass
import concourse.tile as tile
from concourse import bacc, mybir
from concourse import bass2jax
from concourse.bass2jax import _bass_exec_p, install_neuronx_cc_hook, partition_id_tensor

B, T, C = 2, 2048, 1024
H = 16
D = C // H            # 64
SCALE = D ** -0.5     # 0.125
N_CORES = 8
HG = H // (N_CORES // B)   # heads per core = 4
HC = HG * D                # channels per core = 256
KT = C // 128              # 8 contraction tiles
NT = T // 128              # 16 row tiles
NJ = T // 512              # 4 tq tiles
F32 = mybir.dt.float32
F32R = mybir.dt.float32r
BF16 = mybir.dt.bfloat16


# ---------------------------------------------------------------- device code

def _build_nc(block_info, n_uniq, loop_n=None, phases="BCD", cast_dma=False):
    """block_info[j][i] = None (skip) | -1 (full) | ("st", slot) (causal
    staircase applied as a PE matmul accumulation) | ("mk", idx>=0) (general
    mask tile multiplied on DVE)."""
    n_shift = 1 + max([e[1] for row in block_info for e in row
                       if isinstance(e, tuple) and e[0] == "st"], default=-1)
    nc = bacc.Bacc("TRN2", target_bir_lowering=False, debug=False,
                   num_devices=N_CORES)
    xT_ap = nc.dram_tensor("xT", [C, T], BF16, kind="ExternalInput").ap()
    # weights host-packed so each SBUF partition's bytes are contiguous in
    # DRAM (large descriptors; scattered 1KB descriptors measured ~4x slower)
    wqT_ap = nc.dram_tensor("wqT", [128, KT * HC], BF16, kind="ExternalInput").ap()
    wkT_ap = nc.dram_tensor("wkT", [128, KT * HC], BF16, kind="ExternalInput").ap()
    wvT_ap = nc.dram_tensor("wvT", [128, KT * HC], BF16, kind="ExternalInput").ap()
    woT_ap = nc.dram_tensor("woT", [128, 2 * C], BF16, kind="ExternalInput").ap()
    if n_uniq > 0:
        mk_ap = nc.dram_tensor("mk", [128, n_uniq * 512], F32,
                               kind="ExternalInput").ap()
    if n_shift > 0:
        triA_ap = nc.dram_tensor("triA", [128, 128], BF16,
                                 kind="ExternalInput").ap()
        bsh_ap = nc.dram_tensor("bsh", [128, n_shift * 512], BF16,
                                kind="ExternalInput").ap()
    y_ap = nc.dram_tensor("y", [T, C], BF16, kind="ExternalOutput").ap()

    with tile.TileContext(nc) as tc:
        with (
            nc.allow_low_precision(reason="bf16 matmul pipeline"),
            tc.tile_pool(name="glob", bufs=1) as pg,
            tc.tile_pool(name="warm", bufs=1) as pwarm,
            tc.tile_pool(name="ab", bufs=1) as pab,
            tc.tile_pool(name="wp", bufs=2) as pwp,
            tc.tile_pool(name="cd", bufs=1) as pcd,
            tc.tile_pool(name="pt", bufs=4) as ppt,
            tc.tile_pool(name="small", bufs=4) as psm,
            tc.tile_pool(name="ys", bufs=3) as pys,
            # PSUM: 2x [128,2,512] (4 banks) + 4x [128,512] (4 banks)
            tc.tile_pool(name="pst", bufs=2, space="PSUM") as pst,
            tc.tile_pool(name="ppv", bufs=4, space="PSUM") as ppv,
        ):
            # persistent across phases
            qT = pg.tile([128, 2, T], BF16)        # [o-part, o-tile, t]
            # k^T zero-padded per head: head h lives in partition rows
            # 64*(h%2)..+64 of kTz[:, h, :], other rows stay 0 so the score
            # matmul contracts K=128 against the 2-head-packed qT.
            kTz = pg.tile([128, HG, T], BF16)
            # v natural per tk-tile/head, padded to 128 cols: [v | 1 | zeros]
            # (128 cols keeps FWL weight loads; ones col gives softmax denom)
            vaug = pg.tile([128, NT, HG, 128], BF16)
            ones128 = pg.tile([128, 128], BF16)    # all-ones lhsT for bcast
            recipz = pg.tile([128, 512], BF16)     # row0=recip, rows1-127 zero
            ident = pg.tile([128, 128], F32)       # PE-transpose identity
            if n_shift > 0:
                triA = pg.tile([128, 128], BF16)
                bsh = pg.tile([128, n_shift, 512], BF16)
                nc.sync.dma_start(triA[:], triA_ap)
                nc.sync.dma_start(
                    bsh[:], bsh_ap.rearrange("p (s f) -> p s f", f=512))

            # warm the Exp table while DMAs run; zeros/ones are produced via
            # casting copies from f32 tiles (works for every dtype).
            wtile = pwarm.tile([1, 16], F32)
            nc.vector.memset(wtile[:], 0.0)
            nc.scalar.activation(wtile[:], wtile[:],
                                 mybir.ActivationFunctionType.Exp)
            zt = pwarm.tile([128, 512], F32)
            nc.vector.memset(zt[:], 0.0)
            nc.vector.tensor_copy(kTz[:].rearrange("p h (j f) -> p h j f", f=512), zt[:, None, None, :].broadcast_to([128, HG, NJ, 512]))
            nc.vector.tensor_copy(vaug[:], zt[:, None, None, 0:128].broadcast_to([128, NT, HG, 128]))
            nc.vector.tensor_copy(recipz[:], zt[:])
            import concourse.masks as _masks
            _masks.make_identity(nc, ident[:])
            ot = pwarm.tile([128, 16], F32)
            nc.vector.memset(ot[:], 1.0)
            nc.vector.tensor_copy(vaug[:, :, :, D], ot[:, None, 0:HG].broadcast_to([128, NT, HG]))
            nc.vector.tensor_copy(ones128[:], ot[:, 0:1].broadcast_to([128, 128]))

            # persistent phase tiles (hoisted so the loop can pipeline)
            xT = pab.tile([128, KT, T], BF16)
            vT = pab.tile([128, 2, T], F32R)
            woT = pcd.tile([128, 2, C], BF16)
            attnoutT = pcd.tile([128, 2, T], BF16)
            if n_uniq > 0:
                mks = pcd.tile([128, n_uniq, 512], F32)

            def body(it):
                # ---------------- phase A/B: loads + projections ------------
                # chunked so projections start as soon as chunk 0 lands
                for kc in range(KT):
                    nc.sync.dma_start(
                        xT[:, kc, :],
                        xT_ap.rearrange("(k p) t -> k p t", p=128)[kc])
                nc.sync.dma_start(woT[:], woT_ap.rearrange("p (k m) -> p k m", k=2))
                if n_uniq > 0:
                    nc.sync.dma_start(mks[:], mk_ap.rearrange("p (u f) -> p u f", f=512))
                if "B" not in phases:
                    return

                # q^T, k^T, v^T: [o, t] = w_g @ x^T; kc-inner per j-pair so
                # each psum pair drains while the next pair accumulates
                for w_ap, qk in ((wqT_ap, 0), (wkT_ap, 1), (wvT_ap, 2)):
                    w_t = pwp.tile([128, KT, HC], BF16, tag="w", name=f"w{qk}_{it}")
                    nc.sync.dma_start(w_t[:], w_ap.rearrange("p (k m) -> p k m", k=KT))
                    for m in range(2):
                        for jh in range(2):
                            pss = pst.tile([128, 2, 512], F32, tag="st",
                                           name=f"qkps{qk}_{m}_{jh}_{it}")
                            for kc in range(KT):
                                for j2 in range(2):
                                    j = 2 * jh + j2
                                    nc.tensor.matmul(
                                        pss[:, j2, :],
                                        w_t[:, kc, 128 * m:128 * (m + 1)],
                                        xT[:, kc, 512 * j:512 * (j + 1)],
                                        start=(kc == 0), stop=(kc == KT - 1))
                            for j2 in range(2):
                                j = 2 * jh + j2
                                sl = slice(512 * j, 512 * (j + 1))
                                if qk == 0:
                                    nc.scalar.copy(qT[:, m, sl], pss[:, j2, :])
                                elif qk == 2:
                                    nc.scalar.copy(vT[:, m, sl], pss[:, j2, :])
                                else:
                                    # scatter psum head-halves into kTz rows
                                    nc.scalar.copy(kTz[0:64, 2 * m, sl],
                                                   pss[0:64, j2, :])
                                    nc.scalar.copy(kTz[64:128, 2 * m + 1, sl],
                                                   pss[64:128, j2, :])
                # v natural via PE transpose of vT 128x128 blocks
                for m in range(2):
                    for i in range(NT):
                        ps = ppv.tile([128, 512], F32, tag="pv", name=f"vtp{m}_{i}_{it}")
                        nc.tensor.transpose(
                            ps[:, 0:128], vT[:, m, 128 * i:128 * (i + 1)].bitcast(F32),
                            ident[:])
                        nc.vector.tensor_copy(
                            vaug[:, i, 2 * m:2 * m + 2, 0:D],
                            ps[:, 0:128].rearrange("p (h d) -> p h d", h=2))

                # -------- phase C+D: attention, interleaved with out-proj ----
                if "C" not in phases and "D" not in phases:
                    return
                for j in range(NJ):
                    blocks = [(i, bi) for i, bi in enumerate(block_info[j])
                              if bi is not None]
                    chunks = [blocks[c:c + 2] for c in range(0, len(blocks), 2)]
                    for h in range(HG if "C" in phases else 0):
                        m = h // 2
                        jsl = slice(512 * j, 512 * (j + 1))
                        pv = ppv.tile([128, 512], F32, tag="pv", name=f"pv{h}_{j}_{it}")
                        n_acc = len(blocks)
                        acc = 0
                        prev_chunk = None  # (pt, idxs)

                        def emit_pv(pt, idxs):
                            nonlocal acc
                            for c, i in enumerate(idxs):
                                nc.tensor.matmul(
                                    pv[:], vaug[:, i, h, :], pt[:, c, :],
                                    start=(acc == 0), stop=(acc == n_acc - 1))
                                acc += 1

                        for ch in chunks:
                            nsub = len(ch)
                            st = pst.tile([128, 2, 512], F32, tag="st", name=f"st{h}_{j}_{it}")
                            for c, (i, bi) in enumerate(ch):
                                stair = (isinstance(bi, tuple)
                                         and bi[0] == "st")
                                nc.tensor.matmul(
                                    st[:, c, :],
                                    kTz[:, h, 128 * i:128 * (i + 1)],
                                    qT[:, m, jsl],
                                    start=True, stop=not stair)
                                if stair:
                                    # add -98304 to causally-invalid slots
                                    nc.tensor.matmul(
                                        st[:, c, :], triA[:],
                                        bsh[:, bi[1], :],
                                        start=False, stop=True)
                            pt = ppt.tile([128, 2, 512], BF16, tag="pt")
                            # one exp per chunk: the fixed per-op ACT cost
                            # (352 cyc) amortizes over 1024 elements
                            nc.scalar.activation(
                                pt[:, 0:nsub, :], st[:, 0:nsub, :],
                                mybir.ActivationFunctionType.Exp, scale=SCALE)
                            for c, (_, b) in enumerate(ch):
                                if isinstance(b, tuple) and b[0] == "mk":
                                    nc.vector.tensor_mul(
                                        pt[:, c, :], pt[:, c, :],
                                        mks[:, b[1], :])
                            if prev_chunk is not None:
                                emit_pv(*prev_chunk)
                            prev_chunk = (pt, [i for i, _ in ch])
                        emit_pv(*prev_chunk)
                        # normalization: 1/denom (row 64) broadcast down 128
                        # partitions via ones-column matmul; fast approx
                        # recip (18 bits) wants an SBUF f32 operand
                        dnm = psm.tile([1, 512], F32, tag="dnm")
                        nc.vector.tensor_copy(dnm[:], pv[64:65, :])
                        recip = psm.tile([1, 512], F32, tag="recip")
                        nc.vector.reciprocal_approx_fast(recip[:], dnm[:])
                        nc.vector.tensor_copy(recipz[0:1, :], recip[:])
                        bc = ppv.tile([128, 512], F32, tag="pv", name=f"bc{h}_{j}_{it}")
                        nc.tensor.matmul(bc[:], ones128[:], recipz[:],
                                         start=True, stop=True)
                        avu = psm.tile([64, 512], F32, tag="avu")
                        nc.vector.tensor_copy(avu[:], pv[0:64, :])
                        row = 64 * (h % 2)
                        nc.vector.tensor_mul(
                            attnoutT[row:row + 64, m, jsl],
                            avu[:], bc[0:64, :])

                    # ---- phase D for this j: y rows [512j, 512j+512) ----
                    if "D" not in phases:
                        continue
                    for tp in range(2):     # pairs of row tiles
                        ys = pys.tile([128, 2, C], BF16, tag="ys")
                        for tsub in range(2):
                            t = 4 * j + 2 * tp + tsub
                            for o2 in range(2):
                                ps = ppv.tile([128, 512], F32, tag="pv", name=f"yps{t}_{o2}_{it}")
                                for kc in range(2):
                                    nc.tensor.matmul(
                                        ps[:],
                                        attnoutT[:, kc, 128 * t:128 * (t + 1)],
                                        woT[:, kc, 512 * o2:512 * (o2 + 1)],
                                        start=(kc == 0), stop=(kc == 1))
                                nc.vector.tensor_copy(
                                    ys[:, tsub, 512 * o2:512 * (o2 + 1)], ps[:])
                        r0 = 512 * j + 256 * tp
                        nc.scalar.dma_start(
                            y_ap[r0:r0 + 256, :].rearrange("(tt p) o -> p tt o", p=128),
                            ys[:])

            if loop_n is None:
                body(0)
            elif loop_n % 2 == 0:
                # 2x-unrolled loop body: no all-engine reset barrier between
                # the two halves, so half n+1's input DMAs overlap half n
                with tc.For_i(0, loop_n // 2, 1, staggered_reset=True):
                    body(0)
                    body(1)
            else:
                with tc.For_i(0, loop_n, 1, staggered_reset=True):
                    body(0)

    nc.compile()
    return nc


# ---------------------------------------------------------------- run harness

def _install_verbose_hook():
    install_neuronx_cc_hook()
    try:
        import libneuronxla
    except ImportError:
        return
    import traceback
    inner = bass2jax.neuronx_cc_hook

    def wrapped(*a, **kw):
        try:
            return inner(*a, **kw)
        except BaseException:
            traceback.print_exc()
            raise
    libneuronxla.neuronx_cc = wrapped


class _SpmdRunner:
    def __init__(self, nc, n_cores):
        _install_verbose_hook()
        self.nc, self.n_cores = nc, n_cores
        pname = nc.partition_id_tensor.name if nc.partition_id_tensor else None
        in_names, out_names, out_avals = [], [], []
        for alloc in nc.m.functions[0].allocations:
            if not isinstance(alloc, mybir.MemoryLocationSet):
                continue
            name = alloc.memorylocations[0].name
            if alloc.kind == "ExternalInput":
                if name != pname:
                    in_names.append(name)
            elif alloc.kind == "ExternalOutput":
                out_names.append(name)
                out_avals.append(jax.core.ShapedArray(
                    tuple(alloc.tensor_shape), mybir.dt.np(alloc.dtype)))
        self.in_names, self.out_names, self.out_avals = in_names, out_names, out_avals
        n_params = len(in_names)
        all_in = list(in_names) + list(out_names)
        if pname is not None:
            all_in.append(pname)

        def _body(*args):
            operands = list(args)
            if pname is not None:
                operands.append(partition_id_tensor())
            return tuple(_bass_exec_p.bind(
                *operands,
                out_avals=tuple(out_avals), in_names=tuple(all_in),
                out_names=tuple(out_names), lowering_input_output_aliases=(),
                sim_require_finite=True, sim_require_nnan=True, nc=nc))

        devices = jax.devices()[:n_cores]
        self.mesh = Mesh(np.asarray(devices), ("core",))
        in_specs = (PartitionSpec("core"),) * (n_params + len(out_names))
        out_specs = (PartitionSpec("core"),) * len(out_names)
        self.fn = jax.jit(shard_map(_body, mesh=self.mesh, in_specs=in_specs,
                                    out_specs=out_specs, check_rep=False),
                          keep_unused=True)
        self._shard = jax.sharding.NamedSharding(self.mesh, PartitionSpec("core"))

    def put_inputs(self, in_maps):
        arrs = []
        for name in self.in_names:
            cat = np.concatenate([np.asarray(m[name]) for m in in_maps], axis=0)
            arrs.append(jax.device_put(cat, self._shard))
        for av in self.out_avals:
            z = np.zeros((self.n_cores * av.shape[0], *av.shape[1:]), av.dtype)
            arrs.append(jax.device_put(z, self._shard))
        return arrs

    def run(self, dev_args):
        outs = self.fn(*dev_args)
        jax.block_until_ready(outs)
        return outs

    def results(self, outs):
        per_core = []
        for c in range(self.n_cores):
            per_core.append({
                name: np.asarray(outs[i]).reshape(
                    self.n_cores, *self.out_avals[i].shape)[c]
                for i, name in enumerate(self.out_names)})
        return per_core


# ---------------------------------------------------------------- host side

def _mask_blocks(mask):
    """Classify transposed 128x512 blocks of the [T,T] mask.

    Returns (block_info, uniq, shifts) where block_info[j][i] is None (all
    masked), -1 (all valid), ("st", slot) (causal staircase valid = p <=
    f - shifts[slot], applied on-device as a matmul accumulation), or
    ("mk", idx) (arbitrary mixed pattern, multiplied from uniq[idx])."""
    m2 = np.asarray(mask).reshape(T, T)
    valid = (m2 != -np.inf)          # [tq, tk]
    validT = valid.T                 # [tk, tq]
    uniq, keys = [], {}
    shifts, shift_keys = [], {}
    p_idx = np.arange(128)[:, None]
    f_idx = np.arange(512)[None, :]
    block_info = []
    for j in range(NJ):
        row = []
        for i in range(NT):
            blk = validT[128 * i:128 * (i + 1), 512 * j:512 * (j + 1)]
            if not blk.any():
                row.append(None)
                continue
            if blk.all():
                row.append(-1)
                continue
            s = 128 * i - 512 * j
            if -512 < s < 512 and np.array_equal(blk, p_idx <= f_idx - s):
                if s not in shift_keys:
                    shift_keys[s] = len(shifts)
                    shifts.append(s)
                row.append(("st", shift_keys[s]))
                continue
            k = hashlib.sha1(np.ascontiguousarray(blk)).hexdigest()
            if k not in keys:
                keys[k] = len(uniq)
                uniq.append(blk.astype(np.float32))
            row.append(("mk", keys[k]))
        block_info.append(row)
    return block_info, uniq, shifts


_MASK_BIG = -98304.0    # -1.5 * 2**16, exact in bf16


def _stair_operands(shifts):
    """triA [128,128] and bsh [128, n_shift, 512] with
    (triA.T @ bsh[:, slot])[p, f] = _MASK_BIG where p > f - shifts[slot]."""
    k_i = np.arange(128)
    p_i = np.arange(128)
    triA = ((p_i[None, :] > k_i[:, None]) | (k_i[:, None] == 127)
            ).astype(np.float32)
    bsh = np.zeros((128, len(shifts), 512), np.float32)
    for slot, s in enumerate(shifts):
        for f in range(512):
            g = f - s
            if g < 0:
                bsh[127, slot, f] = _MASK_BIG
            elif g <= 126:
                bsh[g, slot, f] = _MASK_BIG
    return triA, bsh


_CACHE = {}


def _get_runner(block_info, n_uniq, loop_n=None, phases="BCD", cast_dma=True):
    key = (str(block_info), n_uniq, loop_n, phases, cast_dma)
    if key not in _CACHE:
        nc = _build_nc(block_info, n_uniq, loop_n=loop_n, phases=phases, cast_dma=cast_dma)
        _CACHE[key] = _SpmdRunner(nc, N_CORES)
    return _CACHE[key]


def _bf16(a):
    return np.ascontiguousarray(np.asarray(a, np.float32)).astype(
        ml_dtypes.bfloat16)


def _pack_rows(a):
    """[R*128, F] -> [128, R*F]: partition-contiguous packing for fast DMA."""
    r = a.shape[0] // 128
    return np.ascontiguousarray(
        a.reshape(r, 128, a.shape[1]).transpose(1, 0, 2).reshape(128, -1))


def _make_in_maps(x, mask, wq, wk, wv, wo):
    block_info, uniq, shifts = _mask_blocks(mask)
    x = np.asarray(x, np.float32)
    extra = {}
    if uniq:
        mk = np.stack(uniq)    # [u,128,512] -> [128, u*512]
        extra["mk"] = np.ascontiguousarray(
            mk.transpose(1, 0, 2).reshape(128, -1))
    if shifts:
        triA, bsh = _stair_operands(shifts)
        extra["triA"] = _bf16(triA)
        extra["bsh"] = _bf16(np.ascontiguousarray(bsh.reshape(128, -1)))
    in_maps = []
    for c in range(N_CORES):
        b, g = c // 4, c % 4
        sl = slice(HC * g, HC * (g + 1))
        in_maps.append({
            "xT": _bf16(x[b].T),
            "wqT": _pack_rows(_bf16(np.asarray(wq)[sl, :].T)),
            "wkT": _pack_rows(_bf16(np.asarray(wk)[sl, :].T)),
            "wvT": _pack_rows(_bf16(np.asarray(wv)[sl, :].T)),
            "woT": _pack_rows(_bf16(np.asarray(wo)[:, sl].T)),
            **extra,
        })
    return in_maps, block_info, len(uniq)


def kernel(x, mask, wq, wk, wv, wo):
    in_maps, block_info, n_uniq = _make_in_maps(x, mask, wq, wk, wv, wo)
    runner = _get_runner(block_info, n_uniq)
    dev = runner.put_inputs(in_maps)
    res = runner.results(runner.run(dev))
    out = np.zeros((B, T, C), np.float32)
    for c in range(N_CORES):
        out[c // 4] += res[c]["y"].astype(np.float32)
    return out


# revision 20
# speedup vs baseline: 1.4688x; 1.0054x over previous
"""Causal multi-head attention (B=2, T=2048, C=1024, H=16, D=64) on 8 trn2 cores.

Sharding: core c -> (batch c//4, head-group c%4 of 4 heads / 256 channels).
Each core computes q/k/v for its head group, causal attention, and a partial
output projection y_part[2048,1024] = attnout_g @ wo_g.T. The host sums the 4
per-group partials of each batch (the "all-reduce after wo" done host-side).

Device kernel (per core, SPMD identical program), bf16 datapath (PE rate is
the same as f32r; bf16 halves SBUF/DMA and enables FWL weight loads):
  phase B: xT,[wq|wk|wv]T loaded as bf16; q^T,k^T (head-channel major, bf16)
           and v (natural, with a ones column -> vaug) via PE matmuls.
  phase C: per head, per tq-tile(512): ST[tk128,tq512] = k^T.T @ q^T; causal
           staircase mask applied INSIDE the score psum group as an extra
           matmul (triA.T @ bsh adds -98304 to invalid slots); P = exp(ST/8)
           on ScalarE (PSUM->SBUF bf16); PV accumulated over tk into
           PSUM[65,512] where row 64 (ones column of vaug) is the softmax
           denominator; normalize with approx-reciprocal + K=1 bcast matmul.
  phase D: y[t,1024] = attnoutT.T @ woT, DMA'd out (bf16) per 256-row tile.

All tile pools live OUTSIDE the timing loop so iteration n+1's input DMAs
overlap iteration n's attention (no SBUF-region reuse between phases).
"""
import time
import hashlib
import numpy as np
import ml_dtypes

import jax
import jax.numpy as jnp
from jax.sharding import Mesh, PartitionSpec
from jax.experimental.shard_map import shard_map

import concourse.bass as bass
import concourse.tile as tile
from concourse import bacc, mybir
from concourse import bass2jax
from concourse.bass2jax import _bass_exec_p, install_neuronx_cc_hook, partition_id_tensor

B, T, C = 2, 2048, 1024
H = 16
D = C // H            # 64
SCALE = D ** -0.5     # 0.125
N_CORES = 8
HG = H // (N_CORES // B)   # heads per core = 4
HC = HG * D                # channels per core = 256
KT = C // 128              # 8 contraction tiles
NT = T // 128              # 16 row tiles
NJ = T // 512              # 4 tq tiles
F32 = mybir.dt.float32
F32R = mybir.dt.float32r
BF16 = mybir.dt.bfloat16


# ---------------------------------------------------------------- device code

def _build_nc(block_info, n_uniq, loop_n=None, phases="BCD", cast_dma=False):
    """block_info[j][i] = None (skip) | -1 (full) | ("st", slot) (causal
    staircase applied as a PE matmul accumulation) | ("mk", idx>=0) (general
    mask tile multiplied on DVE)."""
    n_shift = 1 + max([e[1] for row in block_info for e in row
                       if isinstance(e, tuple) and e[0] == "st"], default=-1)
    nc = bacc.Bacc("TRN2", target_bir_lowering=False, debug=False,
                   num_devices=N_CORES)
    xT_ap = nc.dram_tensor("xT", [C, T], BF16, kind="ExternalInput").ap()
    # weights host-packed so each SBUF partition's bytes are contiguous in
    # DRAM (large descriptors; scattered 1KB descriptors measured ~4x slower)
    wqT_ap = nc.dram_tensor("wqT", [128, KT * HC], BF16, kind="ExternalInput").ap()
    wkT_ap = nc.dram_tensor("wkT", [128, KT * HC], BF16, kind="ExternalInput").ap()
    wvT_ap = nc.dram_tensor("wvT", [128, KT * HC], BF16, kind="ExternalInput").ap()
    woT_ap = nc.dram_tensor("woT", [128, 2 * C], BF16, kind="ExternalInput").ap()
    if n_uniq > 0:
        mk_ap = nc.dram_tensor("mk", [128, n_uniq * 512], F32,
                               kind="ExternalInput").ap()
    if n_shift > 0:
        triA_ap = nc.dram_tensor("triA", [128, 128], BF16,
                                 kind="ExternalInput").ap()
        bsh_ap = nc.dram_tensor("bsh", [128, n_shift * 512], BF16,
                                kind="ExternalInput").ap()
    y_ap = nc.dram_tensor("y", [T, C], BF16, kind="ExternalOutput").ap()

    with tile.TileContext(nc) as tc:
        with (
            nc.allow_low_precision(reason="bf16 matmul pipeline"),
            tc.tile_pool(name="glob", bufs=1) as pg,
            tc.tile_pool(name="warm", bufs=1) as pwarm,
            tc.tile_pool(name="ab", bufs=1) as pab,
            tc.tile_pool(name="wp", bufs=3) as pwp,
            tc.tile_pool(name="cd", bufs=1) as pcd,
            tc.tile_pool(name="pt", bufs=4) as ppt,
            tc.tile_pool(name="small", bufs=4) as psm,
            tc.tile_pool(name="ys", bufs=3) as pys,
            # PSUM: 2x [128,2,512] (4 banks) + 4x [128,512] (4 banks)
            tc.tile_pool(name="pst", bufs=2, space="PSUM") as pst,
            tc.tile_pool(name="ppv", bufs=4, space="PSUM") as ppv,
        ):
            # persistent across phases
            qT = pg.tile([128, 2, T], BF16)        # [o-part, o-tile, t]
            # k^T zero-padded per head: head h lives in partition rows
            # 64*(h%2)..+64 of kTz[:, h, :], other rows stay 0 so the score
            # matmul contracts K=128 against the 2-head-packed qT.
            kTz = pg.tile([128, HG, T], BF16)
            # v natural per tk-tile/head, padded to 128 cols: [v | 1 | zeros]
            # (128 cols keeps FWL weight loads; ones col gives softmax denom)
            vaug = pg.tile([128, NT, HG, 128], BF16)
            ones128 = pg.tile([128, 128], BF16)    # all-ones lhsT for bcast
            recipz = pg.tile([128, 512], BF16)     # row0=recip, rows1-127 zero
            ident = pg.tile([128, 128], F32)       # PE-transpose identity
            if n_shift > 0:
                triA = pg.tile([128, 128], BF16)
                bsh = pg.tile([128, n_shift, 512], BF16)
                nc.sync.dma_start(triA[:], triA_ap)
                nc.sync.dma_start(
                    bsh[:], bsh_ap.rearrange("p (s f) -> p s f", f=512))

            # warm the Exp table while DMAs run; zeros/ones are produced via
            # casting copies from f32 tiles (works for every dtype).
            wtile = pwarm.tile([1, 16], F32)
            nc.vector.memset(wtile[:], 0.0)
            nc.scalar.activation(wtile[:], wtile[:],
                                 mybir.ActivationFunctionType.Exp)
            zt = pwarm.tile([128, 512], F32)
            nc.vector.memset(zt[:], 0.0)
            nc.vector.tensor_copy(kTz[:].rearrange("p h (j f) -> p h j f", f=512), zt[:, None, None, :].broadcast_to([128, HG, NJ, 512]))
            nc.vector.tensor_copy(vaug[:], zt[:, None, None, 0:128].broadcast_to([128, NT, HG, 128]))
            nc.vector.tensor_copy(recipz[:], zt[:])
            import concourse.masks as _masks
            _masks.make_identity(nc, ident[:])
            ot = pwarm.tile([128, 16], F32)
            nc.vector.memset(ot[:], 1.0)
            nc.vector.tensor_copy(vaug[:, :, :, D], ot[:, None, 0:HG].broadcast_to([128, NT, HG]))
            nc.vector.tensor_copy(ones128[:], ot[:, 0:1].broadcast_to([128, 128]))

            # persistent phase tiles (hoisted so the loop can pipeline)
            xT = pab.tile([128, KT, T], BF16)
            vT = pab.tile([128, 2, T], F32R)
            woT = pcd.tile([128, 2, C], BF16)
            attnoutT = pcd.tile([128, 2, T], BF16)
            if n_uniq > 0:
                mks = pcd.tile([128, n_uniq, 512], F32)

            def body(it):
                # ---------------- phase A/B: loads + projections ------------
                # DMA issue order puts wq-half0 + x-chunk0 first so the first
                # projection matmul starts ~3us after the previous body ends
                xr = xT_ap.rearrange("(k p) t -> k p t", p=128)
                w_ts = [pwp.tile([128, KT, HC], BF16, tag="w",
                                 name=f"w{qk}_{it}") for qk in range(3)]
                wqr = wqT_ap.rearrange("p (k m) -> p k m", k=KT)
                nc.sync.dma_start(w_ts[0][:, 0:KT // 2, :], wqr[:, 0:KT // 2, :])
                nc.sync.dma_start(xT[:, 0, :], xr[0])
                nc.sync.dma_start(w_ts[0][:, KT // 2:, :], wqr[:, KT // 2:, :])
                nc.sync.dma_start(xT[:, 1, :], xr[1])
                nc.sync.dma_start(w_ts[1][:], wkT_ap.rearrange("p (k m) -> p k m", k=KT))
                for kc in range(2, KT):
                    nc.sync.dma_start(xT[:, kc, :], xr[kc])
                nc.sync.dma_start(w_ts[2][:], wvT_ap.rearrange("p (k m) -> p k m", k=KT))
                nc.sync.dma_start(woT[:], woT_ap.rearrange("p (k m) -> p k m", k=2))
                if n_uniq > 0:
                    nc.sync.dma_start(mks[:], mk_ap.rearrange("p (u f) -> p u f", f=512))
                if "B" not in phases:
                    return

                # q^T, k^T, v^T: [o, t] = w_g @ x^T; kc-inner per j-pair so
                # each psum pair drains while the next pair accumulates
                for qk in range(3):
                    w_t = w_ts[qk]
                    for m in range(2):
                        for jh in range(2):
                            pss = pst.tile([128, 2, 512], F32, tag="st",
                                           name=f"qkps{qk}_{m}_{jh}_{it}")
                            for kc in range(KT):
                                for j2 in range(2):
                                    j = 2 * jh + j2
                                    nc.tensor.matmul(
                                        pss[:, j2, :],
                                        w_t[:, kc, 128 * m:128 * (m + 1)],
                                        xT[:, kc, 512 * j:512 * (j + 1)],
                                        start=(kc == 0), stop=(kc == KT - 1))
                            for j2 in range(2):
                                j = 2 * jh + j2
                                sl = slice(512 * j, 512 * (j + 1))
                                if qk == 0:
                                    nc.scalar.copy(qT[:, m, sl], pss[:, j2, :])
                                elif qk == 2:
                                    nc.scalar.copy(vT[:, m, sl], pss[:, j2, :])
                                else:
                                    # scatter psum head-halves into kTz rows
                                    nc.scalar.copy(kTz[0:64, 2 * m, sl],
                                                   pss[0:64, j2, :])
                                    nc.scalar.copy(kTz[64:128, 2 * m + 1, sl],
                                                   pss[64:128, j2, :])
                # v natural via PE transpose of vT 128x128 blocks
                for m in range(2):
                    for i in range(NT):
                        ps = ppv.tile([128, 512], F32, tag="pv", name=f"vtp{m}_{i}_{it}")
                        nc.tensor.transpose(
                            ps[:, 0:128], vT[:, m, 128 * i:128 * (i + 1)].bitcast(F32),
                            ident[:])
                        nc.vector.tensor_copy(
                            vaug[:, i, 2 * m:2 * m + 2, 0:D],
                            ps[:, 0:128].rearrange("p (h d) -> p h d", h=2))

                # -------- phase C+D: attention, interleaved with out-proj ----
                if "C" not in phases and "D" not in phases:
                    return
                for j in range(NJ):
                    blocks = [(i, bi) for i, bi in enumerate(block_info[j])
                              if bi is not None]
                    chunks = [blocks[c:c + 2] for c in range(0, len(blocks), 2)]
                    for h in range(HG if "C" in phases else 0):
                        m = h // 2
                        jsl = slice(512 * j, 512 * (j + 1))
                        pv = ppv.tile([128, 512], F32, tag="pv", name=f"pv{h}_{j}_{it}")
                        n_acc = len(blocks)
                        acc = 0
                        prev_chunk = None  # (pt, idxs)

                        def emit_pv(pt, idxs):
                            nonlocal acc
                            for c, i in enumerate(idxs):
                                nc.tensor.matmul(
                                    pv[:], vaug[:, i, h, :], pt[:, c, :],
                                    start=(acc == 0), stop=(acc == n_acc - 1))
                                acc += 1

                        for ch in chunks:
                            nsub = len(ch)
                            st = pst.tile([128, 2, 512], F32, tag="st", name=f"st{h}_{j}_{it}")
                            for c, (i, bi) in enumerate(ch):
                                stair = (isinstance(bi, tuple)
                                         and bi[0] == "st")
                                nc.tensor.matmul(
                                    st[:, c, :],
                                    kTz[:, h, 128 * i:128 * (i + 1)],
                                    qT[:, m, jsl],
                                    start=True, stop=not stair)
                                if stair:
                                    # add -98304 to causally-invalid slots
                                    nc.tensor.matmul(
                                        st[:, c, :], triA[:],
                                        bsh[:, bi[1], :],
                                        start=False, stop=True)
                            pt = ppt.tile([128, 2, 512], BF16, tag="pt")
                            # one exp per chunk: the fixed per-op ACT cost
                            # (352 cyc) amortizes over 1024 elements
                            nc.scalar.activation(
                                pt[:, 0:nsub, :], st[:, 0:nsub, :],
                                mybir.ActivationFunctionType.Exp, scale=SCALE)
                            for c, (_, b) in enumerate(ch):
                                if isinstance(b, tuple) and b[0] == "mk":
                                    nc.vector.tensor_mul(
                                        pt[:, c, :], pt[:, c, :],
                                        mks[:, b[1], :])
                            if prev_chunk is not None:
                                emit_pv(*prev_chunk)
                            prev_chunk = (pt, [i for i, _ in ch])
                        emit_pv(*prev_chunk)
                        # normalization: 1/denom (row 64) broadcast down 128
                        # partitions via ones-column matmul; fast approx
                        # recip (18 bits) wants an SBUF f32 operand
                        dnm = psm.tile([1, 512], F32, tag="dnm")
                        nc.vector.tensor_copy(dnm[:], pv[64:65, :])
                        recip = psm.tile([1, 512], F32, tag="recip")
                        nc.vector.reciprocal_approx_fast(recip[:], dnm[:])
                        nc.vector.tensor_copy(recipz[0:1, :], recip[:])
                        bc = ppv.tile([128, 512], F32, tag="pv", name=f"bc{h}_{j}_{it}")
                        nc.tensor.matmul(bc[:], ones128[:], recipz[:],
                                         start=True, stop=True)
                        avu = psm.tile([64, 512], F32, tag="avu")
                        nc.vector.tensor_copy(avu[:], pv[0:64, :])
                        row = 64 * (h % 2)
                        nc.vector.tensor_mul(
                            attnoutT[row:row + 64, m, jsl],
                            avu[:], bc[0:64, :])

                    # ---- phase D for this j: y rows [512j, 512j+512) ----
                    if "D" not in phases:
                        continue
                    for tp in range(2):     # pairs of row tiles
                        ys = pys.tile([128, 2, C], BF16, tag="ys")
                        for tsub in range(2):
                            t = 4 * j + 2 * tp + tsub
                            for o2 in range(2):
                                ps = ppv.tile([128, 512], F32, tag="pv", name=f"yps{t}_{o2}_{it}")
                                for kc in range(2):
                                    nc.tensor.matmul(
                                        ps[:],
                                        attnoutT[:, kc, 128 * t:128 * (t + 1)],
                                        woT[:, kc, 512 * o2:512 * (o2 + 1)],
                                        start=(kc == 0), stop=(kc == 1))
                                nc.vector.tensor_copy(
                                    ys[:, tsub, 512 * o2:512 * (o2 + 1)], ps[:])
                        r0 = 512 * j + 256 * tp
                        nc.scalar.dma_start(
                            y_ap[r0:r0 + 256, :].rearrange("(tt p) o -> p tt o", p=128),
                            ys[:])

            if loop_n is None:
                body(0)
            else:
                # unrolled loop body: no all-engine reset barrier between the
                # unrolled copies, so copy n+1's input DMAs overlap copy n
                unroll = 4 if loop_n % 4 == 0 else (2 if loop_n % 2 == 0 else 1)
                with tc.For_i(0, loop_n // unroll, 1, staggered_reset=True):
                    for it in range(unroll):
                        body(it)

    nc.compile()
    return nc


# ---------------------------------------------------------------- run harness

def _install_verbose_hook():
    install_neuronx_cc_hook()
    try:
        import libneuronxla
    except ImportError:
        return
    import traceback
    inner = bass2jax.neuronx_cc_hook

    def wrapped(*a, **kw):
        try:
            return inner(*a, **kw)
        except BaseException:
            traceback.print_exc()
            raise
    libneuronxla.neuronx_cc = wrapped


class _SpmdRunner:
    def __init__(self, nc, n_cores):
        _install_verbose_hook()
        self.nc, self.n_cores = nc, n_cores
        pname = nc.partition_id_tensor.name if nc.partition_id_tensor else None
        in_names, out_names, out_avals = [], [], []
        for alloc in nc.m.functions[0].allocations:
            if not isinstance(alloc, mybir.MemoryLocationSet):
                continue
            name = alloc.memorylocations[0].name
            if alloc.kind == "ExternalInput":
                if name != pname:
                    in_names.append(name)
            elif alloc.kind == "ExternalOutput":
                out_names.append(name)
                out_avals.append(jax.core.ShapedArray(
                    tuple(alloc.tensor_shape), mybir.dt.np(alloc.dtype)))
        self.in_names, self.out_names, self.out_avals = in_names, out_names, out_avals
        n_params = len(in_names)
        all_in = list(in_names) + list(out_names)
        if pname is not None:
            all_in.append(pname)

        def _body(*args):
            operands = list(args)
            if pname is not None:
                operands.append(partition_id_tensor())
            return tuple(_bass_exec_p.bind(
                *operands,
                out_avals=tuple(out_avals), in_names=tuple(all_in),
                out_names=tuple(out_names), lowering_input_output_aliases=(),
                sim_require_finite=True, sim_require_nnan=True, nc=nc))

        devices = jax.devices()[:n_cores]
        self.mesh = Mesh(np.asarray(devices), ("core",))
        in_specs = (PartitionSpec("core"),) * (n_params + len(out_names))
        out_specs = (PartitionSpec("core"),) * len(out_names)
        self.fn = jax.jit(shard_map(_body, mesh=self.mesh, in_specs=in_specs,
                                    out_specs=out_specs, check_rep=False),
                          keep_unused=True)
        self._shard = jax.sharding.NamedSharding(self.mesh, PartitionSpec("core"))

    def put_inputs(self, in_maps):
        arrs = []
        for name in self.in_names:
            cat = np.concatenate([np.asarray(m[name]) for m in in_maps], axis=0)
            arrs.append(jax.device_put(cat, self._shard))
        for av in self.out_avals:
            z = np.zeros((self.n_cores * av.shape[0], *av.shape[1:]), av.dtype)
            arrs.append(jax.device_put(z, self._shard))
        return arrs

    def run(self, dev_args):
        outs = self.fn(*dev_args)
        jax.block_until_ready(outs)
        return outs

    def results(self, outs):
        per_core = []
        for c in range(self.n_cores):
            per_core.append({
                name: np.asarray(outs[i]).reshape(
                    self.n_cores, *self.out_avals[i].shape)[c]
                for i, name in enumerate(self.out_names)})
        return per_core


# ---------------------------------------------------------------- host side

def _mask_blocks(mask):
    """Classify transposed 128x512 blocks of the [T,T] mask.

    Returns (block_info, uniq, shifts) where block_info[j][i] is None (all
    masked), -1 (all valid), ("st", slot) (causal staircase valid = p <=
    f - shifts[slot], applied on-device as a matmul accumulation), or
    ("mk", idx) (arbitrary mixed pattern, multiplied from uniq[idx])."""
    m2 = np.asarray(mask).reshape(T, T)
    valid = (m2 != -np.inf)          # [tq, tk]
    validT = valid.T                 # [tk, tq]
    uniq, keys = [], {}
    shifts, shift_keys = [], {}
    p_idx = np.arange(128)[:, None]
    f_idx = np.arange(512)[None, :]
    block_info = []
    for j in range(NJ):
        row = []
        for i in range(NT):
            blk = validT[128 * i:128 * (i + 1), 512 * j:512 * (j + 1)]
            if not blk.any():
                row.append(None)
                continue
            if blk.all():
                row.append(-1)
                continue
            s = 128 * i - 512 * j
            if -512 < s < 512 and np.array_equal(blk, p_idx <= f_idx - s):
                if s not in shift_keys:
                    shift_keys[s] = len(shifts)
                    shifts.append(s)
                row.append(("st", shift_keys[s]))
                continue
            k = hashlib.sha1(np.ascontiguousarray(blk)).hexdigest()
            if k not in keys:
                keys[k] = len(uniq)
                uniq.append(blk.astype(np.float32))
            row.append(("mk", keys[k]))
        block_info.append(row)
    return block_info, uniq, shifts


_MASK_BIG = -98304.0    # -1.5 * 2**16, exact in bf16


def _stair_operands(shifts):
    """triA [128,128] and bsh [128, n_shift, 512] with
    (triA.T @ bsh[:, slot])[p, f] = _MASK_BIG where p > f - shifts[slot]."""
    k_i = np.arange(128)
    p_i = np.arange(128)
    triA = ((p_i[None, :] > k_i[:, None]) | (k_i[:, None] == 127)
            ).astype(np.float32)
    bsh = np.zeros((128, len(shifts), 512), np.float32)
    for slot, s in enumerate(shifts):
        for f in range(512):
            g = f - s
            if g < 0:
                bsh[127, slot, f] = _MASK_BIG
            elif g <= 126:
                bsh[g, slot, f] = _MASK_BIG
    return triA, bsh


_CACHE = {}


def _get_runner(block_info, n_uniq, loop_n=None, phases="BCD", cast_dma=True):
    key = (str(block_info), n_uniq, loop_n, phases, cast_dma)
    if key not in _CACHE:
        nc = _build_nc(block_info, n_uniq, loop_n=loop_n, phases=phases, cast_dma=cast_dma)
        _CACHE[key] = _SpmdRunner(nc, N_CORES)
    return _CACHE[key]


def _bf16(a):
    return np.ascontiguousarray(np.asarray(a, np.float32)).astype(
        ml_dtypes.bfloat16)


def _pack_rows(a):
    """[R*128, F] -> [128, R*F]: partition-contiguous packing for fast DMA."""
    r = a.shape[0] // 128
    return np.ascontiguousarray(
        a.reshape(r, 128, a.shape[1]).transpose(1, 0, 2).reshape(128, -1))


def _make_in_maps(x, mask, wq, wk, wv, wo):
    block_info, uniq, shifts = _mask_blocks(mask)
    x = np.asarray(x, np.float32)
    extra = {}
    if uniq:
        mk = np.stack(uniq)    # [u,128,512] -> [128, u*512]
        extra["mk"] = np.ascontiguousarray(
            mk.transpose(1, 0, 2).reshape(128, -1))
    if shifts:
        triA, bsh = _stair_operands(shifts)
        extra["triA"] = _bf16(triA)
        extra["bsh"] = _bf16(np.ascontiguousarray(bsh.reshape(128, -1)))
    in_maps = []
    for c in range(N_CORES):
        b, g = c // 4, c % 4
        sl = slice(HC * g, HC * (g + 1))
        in_maps.append({
            "xT": _bf16(x[b].T),
            "wqT": _pack_rows(_bf16(np.asarray(wq)[sl, :].T)),
            "wkT": _pack_rows(_bf16(np.asarray(wk)[sl, :].T)),
            "wvT": _pack_rows(_bf16(np.asarray(wv)[sl, :].T)),
            "woT": _pack_rows(_bf16(np.asarray(wo)[:, sl].T)),
            **extra,
        })
    return in_maps, block_info, len(uniq)


def kernel(x, mask, wq, wk, wv, wo):
    in_maps, block_info, n_uniq = _make_in_maps(x, mask, wq, wk, wv, wo)
    runner = _get_runner(block_info, n_uniq)
    dev = runner.put_inputs(in_maps)
    res = runner.results(runner.run(dev))
    out = np.zeros((B, T, C), np.float32)
    for c in range(N_CORES):
        out[c // 4] += res[c]["y"].astype(np.float32)
    return out
